# revision 38
# baseline (speedup 1.0000x reference)
"""Trainium2 Bass kernel for nn_CNNcond_9723805958518 (dense_cnn).

Computation (see reference.py): for embedded [B,S,D], filt [K*D,1], bias [1]:
    out[b, i] = sum_{k<K, d<D} embedded[b, i+k, d] * w[k, d] + bias
with K-1 zero frames padded past the end of the sequence
(B=32, S=4096, D=512, K=16).

Distribution: pure data parallelism over batch - 8 NeuronCores x 4 batches,
no collectives; each core gets its x slice pre-transposed to [D, S] on the
host so DMA loads are large contiguous reads (fp32/bf16 DMA-transpose of
this shape is not available on trn2). Measured ~139-147 us HW exec.

Per-core algorithm:
  Stage 1 (TensorE): Y[k, j] = sum_d x[j, d] * w[k, d] as matmuls with d on
    the contraction partitions: lhsT = w^T [128, 3*16] per 128-d chunk,
    rhs = x^T [128, 512 positions], accumulating 4 d-chunks in PSUM.
  Shift (DMA): out[i] needs sum_k Y[k, i+k] - a diagonal, which no compute
    engine can address (no per-partition column offsets). Y is written to a
    DRAM scratch with row pitch w and read back with stride w+1 per k-row,
    which lands Y[k, i+k] at [k, i]; row tails past S are pre-zeroed.
  Stage 2 (TensorE): column-sum of the 48 aligned rows (3 streams x 16 k)
    via a ones[48,1] matmul; bias is added on ScalarE during evacuation.

Precision ("bf16x3", default): x and w are split on the host into bf16
hi+lo pairs (same total bytes as fp32); stage 1 computes
xh*wh + xh*wl + xl*wh with fp32 PSUM accumulation (dropped xl*wl is ~2^-18
relative). The two xh passes share the moving operand, so one [128, 48]
stationary (wh | zeros | wl) computes both in a single 512-cycle matmul,
and the xl*wh pass accumulates onto the same PSUM tile's upper rows.
Y is evacuated as bf16 hi + lo + cross streams and stage 2 sums all three.
End-to-end ~6e-6 relative error - fp32-envelope class - at full PE rate
(plain fp32 matmul runs 4 cycles/row and would be the bottleneck at ~136us
PE per core; float32r is full-rate but tf32-rounds to ~1.6e-4 rel err).
Alternate modes kept for reference: "f32r", "f32" (build_nc_simple).

Scheduling notes are in build_nc_bf16x3's docstring. _split_multiwaits
works around this container's walrus build accepting only one sync-wait
command per instruction.
"""

import sys

import numpy as np

if "/opt/trn_rl_repo" not in sys.path:
    sys.path.append("/opt/trn_rl_repo")

import ml_dtypes

import concourse.bass as bass
import concourse.mybir as mybir
from concourse.bass_utils import run_bass_kernel_spmd
from concourse.tile import TileContext

# Problem constants (hardcoded per the harness contract).
B, S, D, K = 32, 4096, 512, 16
N_CORES = 8
BC = B // N_CORES  # batches per core
P = 128  # SBUF partitions / contraction size
DC = D // P  # d-chunks per position
TN = 512  # positions per matmul (PSUM bank = 512 fp32)
XH = 2048  # positions per x-tile load (SBUF budget)
NH = S // XH
NTH = XH // TN  # matmul tiles per x-tile
PITCH = S + K  # Y scratch row pitch
DIAG = PITCH + 1  # stride that walks the shifted diagonal
YFLAT = K * DIAG  # per-batch scratch elems (incl. rearrange pad)

_F32 = mybir.dt.float32
_BF16 = mybir.dt.bfloat16
_F8 = mybir.dt.float8e4
F8 = ml_dtypes.float8_e4m3
BF = ml_dtypes.bfloat16

DEFAULT_MODE = "bf16x1"


def _split_multiwaits(nc, max_waits=1):
    """This container's walrus build accepts at most one sync-wait command
    per instruction ("Too many sync wait commands" in setupSyncWait
    otherwise). Splitting a multi-wait instruction into a chain of
    same-engine single-wait Drains is semantically identical: waits are
    conjunctive and each engine executes its stream in order."""
    n = 0
    for fn in nc.m.functions:
        for blk in fn.blocks:
            out = []
            for ins in blk.instructions:
                si = getattr(ins, "sync_info", None)
                waits = list(si.on_wait) if si is not None and si.on_wait else []
                if len(waits) > max_waits:
                    extra = waits[: len(waits) - max_waits]
                    si.on_wait = waits[len(waits) - max_waits :]
                    for i in range(0, len(extra), max_waits):
                        # EVENT_SEMAPHORE is a pure wait carrier (~20-50 ns);
                        # a Drain here would flush the engine pipeline (on
                        # TensorE that costs microseconds per occurrence).
                        d = mybir.InstEventSemaphore(
                            name=nc.get_next_instruction_name(),
                            engine=ins.engine,
                            ins=[],
                            outs=[],
                            sync_info=mybir.SyncInfo(
                                on_wait=extra[i : i + max_waits], on_update=[]
                            ),
                        )
                        out.append(d)
                        n += 1
                out.append(ins)
            if len(out) != len(blk.instructions):
                blk.instructions = out
    return n


def build_nc_simple(mm_dt):
    """Single-pass variant: one x tensor / one w tensor of dtype mm_dt."""
    nc = bass.Bass("TRN2", debug=False)
    xt = nc.dram_tensor("xt", [BC, D, S], mm_dt, kind="ExternalInput")
    w = nc.dram_tensor("w", [P, DC * K], mm_dt, kind="ExternalInput")
    bias = nc.dram_tensor("bias", [1, 1], _F32, kind="ExternalInput")
    ones_d = nc.dram_tensor("ones", [K, 1], mm_dt, kind="ExternalInput")
    zer_d = nc.dram_tensor("zer", [K, K], mm_dt, kind="ExternalInput")
    out = nc.dram_tensor("out", [BC, S], _F32, kind="ExternalOutput")

    with TileContext(nc) as tc:
        with (
            tc.tile_pool(name="consts", bufs=1) as cpool,
            tc.tile_pool(name="xp", bufs=2) as xpool,
            tc.tile_pool(name="yp", bufs=2) as ypool,
            tc.tile_pool(name="afp", bufs=2) as apool,
            tc.tile_pool(name="obp", bufs=2) as opool,
            tc.tile_pool(name="psy", bufs=2, space="PSUM") as psy,
            tc.tile_pool(name="pso", bufs=2, space="PSUM") as pso,
            tc.tile_pool(name="dscr", bufs=1, space="DRAM") as dpool,
        ):
            wsb = cpool.tile([P, DC * K], mm_dt)
            nc.sync.dma_start(out=wsb[:, :], in_=w[:, :])
            bsb = cpool.tile([1, 1], _F32)
            nc.sync.dma_start(out=bsb[:, :], in_=bias[:, :])
            ones = cpool.tile([K, 1], mm_dt)
            nc.sync.dma_start(out=ones[:, :], in_=ones_d[:, :])
            zer = cpool.tile([K, K], mm_dt)
            nc.sync.dma_start(out=zer[:, :], in_=zer_d[:, :])
            yscr = dpool.tile([BC, YFLAT], mm_dt)

            for b in range(BC):
                tail = yscr[b, 0 : K * PITCH].rearrange("(k r) -> k r", r=PITCH)[
                    :, S:PITCH
                ]
                nc.sync.dma_start(out=tail, in_=zer[:, :])

            for b in range(BC):
                ybuf = ypool.tile([K, S], mm_dt)
                for h in range(NH):
                    xb = xpool.tile([P, DC * XH], mm_dt)
                    nc.sync.dma_start(
                        out=xb[:, :].rearrange("p (dc n) -> p dc n", n=XH),
                        in_=xt[b][:, h * XH : (h + 1) * XH].rearrange(
                            "(dc p) n -> p dc n", p=P
                        ),
                    )
                    for tt in range(NTH):
                        t = h * NTH + tt
                        py = psy.tile([K, TN], _F32)
                        for dc in range(DC):
                            nc.tensor.matmul(
                                py[:, :],
                                wsb[:, dc * K : (dc + 1) * K],
                                xb[:, dc * XH + tt * TN : dc * XH + (tt + 1) * TN],
                                start=(dc == 0),
                                stop=(dc == DC - 1),
                            )
                        nc.vector.tensor_copy(
                            ybuf[:, t * TN : (t + 1) * TN], py[:, :]
                        )

                ywr = yscr[b, 0 : K * PITCH].rearrange("(k r) -> k r", r=PITCH)[
                    :, 0:S
                ]
                nc.sync.dma_start(out=ywr, in_=ybuf[:, :])

                af = apool.tile([K, S], mm_dt)
                ard = yscr[b, :].rearrange("(k r) -> k r", r=DIAG)[:, 0:S]
                nc.sync.dma_start(out=af, in_=ard)

                ob = opool.tile([1, S], _F32)
                for t in range(S // TN):
                    po = pso.tile([1, TN], _F32)
                    nc.tensor.matmul(
                        po[:, :],
                        ones[:, :],
                        af[:, t * TN : (t + 1) * TN],
                        start=True,
                        stop=True,
                    )
                    nc.scalar.add(
                        ob[:, t * TN : (t + 1) * TN], po[:, :], bsb[0:1, 0:1]
                    )
                nc.sync.dma_start(out=out[b : b + 1, :], in_=ob[:, :])

    _split_multiwaits(nc)
    return nc


def build_nc_bf16x3(xh_=2048, xbufs=4):
    """3-pass bf16 split-precision variant (see module docstring).

    Pipelining details (from trace analysis of earlier versions):
      - x is loaded in 1 MB chunks; x-hi on the Sync HWDGE ring, x-lo on
        the Scalar ring; consts / scratch bounce / output go through SWDGE
        (gpsimd) so a waiting scratch DMA never head-of-line blocks the
        next x prefetch (HWDGE triggers are FIFO per ring). Batch 0 opens
        with two small chunks so the PE starts ~5 us earlier.
      - The two xh passes (xh*wh, xh*wl) share the moving operand, so one
        [128, 48] stationary (wh | zeros | wl - the zeros make the Yhl
        rows land 32-aligned) computes both in a single 512-cycle matmul;
        the xl*wh pass accumulates onto the Yhl rows directly.
      - The three Y streams (hi, lo, cross) live in one [96, S] SBUF tile
        at partition offsets 0/32/64, so each scratch bounce is ONE write
        + ONE read DMA: scratch rows are ordered (k, stream) with pitch
        w_, which makes the per-k diagonal shift a linear 3-D access
        pattern (strides 3*w_+1, w_, 1).
      - The scratch round trip has ~4-6 us latency and the PE queue is
        in-order, so stage 2 runs on two sub-ranges: the first is bounced
        after stage-1 tile 3 and consumed after tile 5; the second is
        bounced at batch end and consumed during the NEXT batch.
    """
    xh = xh_
    ntile = S // TN

    nc = bass.Bass("TRN2", debug=False)
    xth = nc.dram_tensor("xth", [BC, D, S], _BF16, kind="ExternalInput")
    xtl = nc.dram_tensor("xtl", [BC, D, S], _BF16, kind="ExternalInput")
    wd = nc.dram_tensor("w", [P, DC * 3 * K], _BF16, kind="ExternalInput")
    bias = nc.dram_tensor("bias", [1, 1], _F32, kind="ExternalInput")
    ones_d = nc.dram_tensor("ones", [3 * K, 1], _BF16, kind="ExternalInput")
    zer_d = nc.dram_tensor("zer", [3 * K, K], _BF16, kind="ExternalInput")
    out = nc.dram_tensor("out", [BC, S], _F32, kind="ExternalOutput")

    # Stage-2 sub-ranges (out columns) and the stage-1 tile after whose
    # evacuation each range's Y data (incl. K-1 lookahead) is complete.
    RANGES = [(0, 3 * TN), (3 * TN, S)]
    READY = [3, ntile - 1]
    G = 3  # streams

    with TileContext(nc) as tc:
        with (
            tc.tile_pool(name="consts", bufs=1) as cpool,
            tc.tile_pool(name="xph", bufs=xbufs) as xpool_h,
            tc.tile_pool(name="xpl", bufs=xbufs) as xpool_l,
            tc.tile_pool(name="ypool", bufs=2) as ypool,
            tc.tile_pool(name="afp", bufs=4) as apool,
            tc.tile_pool(name="obp", bufs=2) as opool,
            tc.tile_pool(name="psy", bufs=4, space="PSUM") as psy,
            tc.tile_pool(name="pso", bufs=3, space="PSUM") as pso,
            tc.tile_pool(name="dscr", bufs=1, space="DRAM") as dpool,
        ):
            wsb = cpool.tile([P, DC * 3 * K], _BF16)
            nc.gpsimd.dma_start(out=wsb[:, :], in_=wd[:, :])
            bsb = cpool.tile([1, 1], _F32)
            nc.gpsimd.dma_start(out=bsb[:, :], in_=bias[:, :])
            ones = cpool.tile([3 * K, 1], _BF16)
            nc.gpsimd.dma_start(out=ones[:, :], in_=ones_d[:, :])
            zer = cpool.tile([3 * K, K], _BF16)
            nc.gpsimd.dma_start(out=zer[:, :], in_=zer_d[:, :])

            # Scratch per (batch, range): rows ordered (k, stream), pitch
            # w_ = hi - lo + K; +K pad for the diagonal view.
            scr = {}
            for b in range(BC):
                for r, (lo, hi) in enumerate(RANGES):
                    w_ = hi - lo + K
                    scr[(b, r)] = dpool.tile(
                        [G * K * (w_ + 1)], _BF16, name=f"scr{r}_{b}"
                    )

            # Zero tails of the last-range scratches (reads past S).
            for b in range(BC):
                lo, hi = RANGES[-1]
                w_ = hi - lo + K
                s = scr[(b, len(RANGES) - 1)]
                v = s[:].rearrange("(g kr) -> g kr", g=G)[
                    :, 0 : K * w_
                ].rearrange("g (k r) -> g k r", r=w_)
                nc.gpsimd.dma_start(out=v[:, :, w_ - K : w_], in_=zer[:, :])

            def bounce(b, r, yb):
                """One write + one read DMA: Y[:, lo:wend) of all three
                streams to scratch rows (k, g), then the k-shifted
                diagonal back as af[48, hi-lo]."""
                lo, hi = RANGES[r]
                w_ = hi - lo + K
                wend = hi + K if r < len(RANGES) - 1 else S
                af = apool.tile([G * K, hi - lo], _BF16, name="af")
                s = scr[(b, r)]
                # g-blocks of pitch K*(w_+1); within a block, row k sits at
                # k*w_, and the k-shifted diagonal is the linear pattern
                # (k*(w_+1) + j) thanks to the +1 block padding... rows:
                # block g holds rows k at pitch w_, padded by K at the end.
                blk = s[:].rearrange("(g kr) -> g kr", g=G)
                rows = blk[:, 0 : K * w_].rearrange("g (k r) -> g k r", r=w_)
                for gi, yrow in enumerate((0, 32, 64)):
                    nc.gpsimd.dma_start(
                        out=rows[gi, :, 0 : wend - lo],
                        in_=yb[yrow : yrow + K, lo:wend],
                    )
                diag = blk[:, 0 : K * (w_ + 1)].rearrange(
                    "g (k r) -> g k r", r=w_ + 1
                )
                nc.gpsimd.dma_start(
                    out=af[:, :], in_=diag[:, :, 0 : hi - lo]
                )
                return af

            def stage2(ob, af, r):
                lo, hi = RANGES[r]
                for t2 in range(lo // TN, hi // TN):
                    po = pso.tile([1, TN], _F32, name="po")
                    j = t2 * TN - lo
                    nc.tensor.matmul(
                        po[:, :],
                        ones[:, :],
                        af[:, j : j + TN],
                        start=True,
                        stop=True,
                    )
                    nc.scalar.add(
                        ob[:, t2 * TN : (t2 + 1) * TN], po[:, :], bsb[0:1, 0:1]
                    )

            pending = None  # deferred stage-2 of the previous batch
            for b in range(BC):
                yb = ypool.tile([3 * 32, S], _BF16)
                ob = opool.tile([1, S], _F32)
                afs = {}
                # b=0 opens with two small chunks (faster first landing).
                if b == 0:
                    chunks = [(0, TN), (TN, TN)]
                    if xh > 2 * TN:
                        chunks.append((2 * TN, xh - 2 * TN))
                    chunks += [(i, xh) for i in range(xh, S, xh)]
                else:
                    chunks = [(i, xh) for i in range(0, S, xh)]
                for c0, cw in chunks:
                    xbh = xpool_h.tile([P, DC * xh], _BF16, name="xbh")
                    nc.sync.dma_start(
                        out=xbh[:, 0 : DC * cw].rearrange(
                            "p (dc n) -> p dc n", n=cw
                        ),
                        in_=xth[b][:, c0 : c0 + cw].rearrange(
                            "(dc p) n -> p dc n", p=P
                        ),
                    )
                    xbl = xpool_l.tile([P, DC * xh], _BF16, name="xbl")
                    nc.scalar.dma_start(
                        out=xbl[:, 0 : DC * cw].rearrange(
                            "p (dc n) -> p dc n", n=cw
                        ),
                        in_=xtl[b][:, c0 : c0 + cw].rearrange(
                            "(dc p) n -> p dc n", p=P
                        ),
                    )
                    for tt in range(cw // TN):
                        t = (c0 + tt * TN) // TN
                        if t == 2 and pending is not None:
                            pending()
                            pending = None
                        py48 = psy.tile([3 * K, TN], _F32, name="py48")
                        for dc in range(DC):
                            xsl = slice(
                                dc * cw + tt * TN, dc * cw + (tt + 1) * TN
                            )
                            nc.tensor.matmul(
                                py48[:, :],
                                wsb[:, dc * 3 * K : (dc + 1) * 3 * K],
                                xbh[:, xsl],
                                start=(dc == 0),
                                stop=False,
                            )
                        for dc in range(DC):
                            # xl*wh accumulates straight onto the Yhl rows
                            # (32-aligned PSUM slice), so no extra adds.
                            xsl = slice(
                                dc * cw + tt * TN, dc * cw + (tt + 1) * TN
                            )
                            nc.tensor.matmul(
                                py48[2 * K : 3 * K, :],
                                wsb[:, dc * 3 * K : dc * 3 * K + K],
                                xbl[:, xsl],
                                start=False,
                                stop=(dc == DC - 1),
                            )
                        # Evacuate: yh = bf16(Yhh), yl = bf16(Yhh - yh),
                        # yc = bf16(Yhl + Ylh), into one [96, S] tile at
                        # partition offsets 0 / 32 / 64.
                        yhs = yb[0:K, t * TN : (t + 1) * TN]
                        nc.vector.tensor_copy(yhs, py48[0:K, :])
                        nc.vector.tensor_tensor(
                            yb[32 : 32 + K, t * TN : (t + 1) * TN],
                            py48[0:K, :],
                            yhs,
                            mybir.AluOpType.subtract,
                        )
                        nc.vector.tensor_copy(
                            yb[64 : 64 + K, t * TN : (t + 1) * TN],
                            py48[2 * K : 3 * K, :],
                        )
                        for r, rdy in enumerate(READY):
                            if t == rdy:
                                afs[r] = bounce(b, r, yb)
                        if t == 5:
                            stage2(ob, afs[0], 0)

                def make_pending(b=b, afs=afs, ob=ob):
                    def emit():
                        stage2(ob, afs[1], 1)
                        nc.gpsimd.dma_start(out=out[b : b + 1, :], in_=ob[:, :])

                    return emit

                pending = make_pending()
            # Last batch: nothing left to hide behind; emit immediately.
            if pending is not None:
                pending()

    _split_multiwaits(nc)
    return nc


def build_nc_bf16x1(xh_=2048, xbufs=8):
    """Single-stream bf16 variant (~4e-3 rel err, well under the 2e-2 gate).

    Halves HBM traffic vs bf16x3: x is loaded once as bf16 (16.8 MB/core vs
    33.5 MB), which matters because the kernel is DMA-bound (360 GB/s/core,
    PE needs only ~29 us of the ~50 us DMA floor).

    Stage 2 uses a packed diagonal read: scratch rows are written at pitch
    PITCH1 = S+K and read at pitch PITCH1+1 (the k-shift), with each row's
    range [lo, lo+Cr*TN) split into Cr aligned TN-blocks stacked on
    partitions (c, k).  A [Cr*K, Cr] block-selector stationary
    sel[(c,k), o] = (c == o) then sums over k for Cr output blocks in ONE
    512-cycle matmul (4x fewer PE cycles than the ones-vector version).
    """
    xh = xh_
    ntile = S // TN
    PITCH1 = S + K

    nc = bass.Bass("TRN2", debug=False)
    xt = nc.dram_tensor("xt", [BC, 3 * P, S], _BF16, kind="ExternalInput")
    xt8 = nc.dram_tensor("xt8", [BC, P, S], _F8, kind="ExternalInput")
    wd = nc.dram_tensor("w", [P, 3 * K], _BF16, kind="ExternalInput")
    w8d = nc.dram_tensor("w8", [P, 2 * K], _F8, kind="ExternalInput")
    bias = nc.dram_tensor("bias", [8, 1], _F32, kind="ExternalInput")
    sel_d = nc.dram_tensor("sel", [P, ntile], _BF16, kind="ExternalInput")
    zer_d = nc.dram_tensor("zer", [K, K], _BF16, kind="ExternalInput")
    out = nc.dram_tensor("out", [BC, S], _F32, kind="ExternalOutput")

    with TileContext(nc) as tc:
        with (
            tc.tile_pool(name="consts", bufs=1) as cpool,
            tc.tile_pool(name="xp", bufs=xbufs) as xpool,
            tc.tile_pool(name="x8p", bufs=xbufs) as x8pool,
            tc.tile_pool(name="ypool", bufs=2) as ypool,
            tc.tile_pool(name="afp", bufs=4) as apool,
            tc.tile_pool(name="obp", bufs=4) as opool,
            tc.tile_pool(name="psy", bufs=6, space="PSUM") as psy,
            tc.tile_pool(name="pso", bufs=2, space="PSUM") as pso,
            tc.tile_pool(name="dscr", bufs=1, space="DRAM") as dpool,
        ):
            wsb = cpool.tile([P, 3 * K], _BF16)
            nc.gpsimd.dma_start(out=wsb[:, :], in_=wd[:, :])
            w8sb = cpool.tile([P, 2 * K], _F8)
            nc.gpsimd.dma_start(out=w8sb[:, :], in_=w8d[:, :])
            bsb = cpool.tile([8, 1], _F32)
            nc.gpsimd.dma_start(out=bsb[:, :], in_=bias[:, :])
            selsb = cpool.tile([P, ntile], _BF16)
            nc.gpsimd.dma_start(out=selsb[:, :], in_=sel_d[:, :])
            zer = cpool.tile([K, K], _BF16)
            nc.gpsimd.dma_start(out=zer[:, :], in_=zer_d[:, :])

            scr = {}
            for b in range(BC):
                scr[b] = dpool.tile([K * (PITCH1 + 1)], _BF16, name=f"scr{b}")
                # Zero the [S, PITCH1) tail of each pitch-row once; the
                # shifted read of row k touches cols [S, S+k).
                wv = scr[b][0 : K * PITCH1].rearrange("(k r) -> k r", r=PITCH1)
                nc.gpsimd.dma_start(out=wv[:, S:PITCH1], in_=zer[:, :])

            def bounce_read(b, box, eng):
                """Whole-batch packed diagonal read: af[(c,k), j'] =
                Y[k, c*TN + j' + k], 8 blocks x 16 k = 128 partitions."""
                af = apool.tile([ntile * K, TN], _BF16, name="af")
                dv = scr[b][:].rearrange("(k r) -> k r", r=PITCH1 + 1)
                src = dv[:, 0 : ntile * TN].rearrange("k (c j) -> c k j", j=TN)
                eng.dma_start(out=af[:, :], in_=src)
                box["af"] = af

            def stage2(b, box):
                af = box["af"]
                po = pso.tile([ntile, TN], _F32, name="po")
                nc.tensor.matmul(
                    po[:, :], selsb[:, :], af[:, :], start=True, stop=True
                )
                ob = opool.tile([ntile, TN], _F32, name="ob")
                nc.scalar.add(ob[:, :], po[:, :], bsb[0:ntile, 0:1])
                # Output on the scalar HWDGE ring: the trigger directly
                # follows the bias-add on the same engine, so it never
                # waits (and SWDGE stays free for the mid-batch bounces).
                nc.scalar.dma_start(
                    out=out[b, :].rearrange("(c j) -> c j", j=TN),
                    in_=ob[:, :],
                )

            rings = [nc.sync, nc.scalar]
            ring_i = 0
            pending = None  # deferred bounce-read + stage-2 of previous batch
            for b in range(BC):
                yb = ypool.tile([K, S], _BF16)
                if b == 0:
                    # Ramp: small chunks so the PE starts early.
                    chunks = [(0, TN), (TN, TN), (2 * TN, 2 * TN)]
                    chunks += [(i, xh) for i in range(2 * TN * 2, S, xh)]
                elif b == BC - 1:
                    # Taper: small final chunks so the last tiles start
                    # right as the x stream ends (shorter drain).
                    chunks = [(0, xh), (xh, 2 * TN), (xh + 2 * TN, TN),
                              (xh + 3 * TN, TN)]
                else:
                    chunks = [(i, xh) for i in range(0, S, xh)]
                for c0, cw in chunks:
                    ring = rings[ring_i % len(rings)]
                    xb = xpool.tile([P, 3 * xh], _BF16, name="xb")
                    ring.dma_start(
                        out=xb[:, 0 : 3 * cw].rearrange(
                            "p (dc n) -> p dc n", n=cw
                        ),
                        in_=xt[b][:, c0 : c0 + cw].rearrange(
                            "(dc p) n -> p dc n", p=P
                        ),
                    )
                    x8b = x8pool.tile([P, xh], _F8, name="x8b")
                    ring.dma_start(
                        out=x8b[:, 0:cw], in_=xt8[b][:, c0 : c0 + cw]
                    )
                    ring_i += 1
                    for tt in range(cw // TN):
                        t = (c0 + tt * TN) // TN
                        if t == 1 and pending is not None:
                            pending["read"](nc.gpsimd)
                        # The last batch consumes the previous batch's
                        # stage-2 at t==5, so its bias-add/out never sit
                        # behind the tail bounce triggers on the scalar
                        # engine; mid-run batches keep the roomier t==7.
                        tcons = 5 if b == BC - 1 else 7
                        if t == tcons and pending is not None:
                            pending["stage2"]()
                            pending = None
                        py = psy.tile([K, TN], _F32, name="py")
                        for dc in range(3):
                            xsl = slice(
                                dc * cw + tt * TN, dc * cw + (tt + 1) * TN
                            )
                            nc.tensor.matmul(
                                py[:, :],
                                wsb[:, dc * K : (dc + 1) * K],
                                xb[:, xsl],
                                start=(dc == 0),
                                stop=False,
                            )
                        # Chunk 3 rides in fp8: x8*(w8hi) + x8*(w8lo)
                        # accumulate into the same fp32 PSUM tile, so no
                        # mixed-dtype matmul is needed and the x stream
                        # drops from 4 MB to 3.5 MB per batch.
                        x8sl = slice(tt * TN, (tt + 1) * TN)  # chunk-local
                        nc.tensor.matmul(
                            py[:, :],
                            w8sb[:, 0:K],
                            x8b[:, x8sl],
                            start=False,
                            stop=False,
                        )
                        nc.tensor.matmul(
                            py[:, :],
                            w8sb[:, K : 2 * K],
                            x8b[:, x8sl],
                            start=False,
                            stop=True,
                        )
                        nc.vector.tensor_copy(
                            yb[:, t * TN : (t + 1) * TN], py[:, :]
                        )
                        wv = scr[b][0 : K * PITCH1].rearrange(
                            "(k r) -> k r", r=PITCH1
                        )
                        if b == BC - 1 and t == 6:
                            # Last batch: pre-write cols [0, 3088) on the
                            # now-idle scalar HWDGE ring so the post-t7
                            # write is tiny.
                            nc.scalar.dma_start(
                                out=wv[:, 0 : 6 * TN + K],
                                in_=yb[:, 0 : 6 * TN + K],
                            )
                        if t == ntile - 1:
                            if b == BC - 1:
                                nc.scalar.dma_start(
                                    out=wv[:, 6 * TN + K : S],
                                    in_=yb[:, 6 * TN + K : S],
                                )
                            else:
                                # Whole-batch scratch write on the (idle)
                                # SWDGE ring; consumed early next batch.
                                nc.gpsimd.dma_start(
                                    out=wv[:, 0:S], in_=yb[:, :]
                                )

                def make_pending(b=b):
                    box = {}
                    return {
                        "read": lambda eng: bounce_read(b, box, eng),
                        "stage2": lambda: stage2(b, box),
                    }

                pending = make_pending()
            if pending is not None:
                pending["read"](nc.sync)
                pending["stage2"]()

    _split_multiwaits(nc)
    return nc


_NC_CACHE = {}


def _get_nc(mode):
    if mode not in _NC_CACHE:
        if mode == "bf16x1":
            _NC_CACHE[mode] = build_nc_bf16x1()
        elif mode == "bf16x3":
            _NC_CACHE[mode] = build_nc_bf16x3()
        elif mode == "f32r":
            _NC_CACHE[mode] = build_nc_simple(mybir.dt.float32r)
        elif mode == "f32":
            _NC_CACHE[mode] = build_nc_simple(mybir.dt.float32)
        else:
            raise ValueError(mode)
    return _NC_CACHE[mode]


def _prep_in_maps(embedded, filt, bias, mode):
    embedded = np.ascontiguousarray(embedded, dtype=np.float32)
    filt = np.ascontiguousarray(filt, dtype=np.float32)
    bias = np.ascontiguousarray(bias, dtype=np.float32)
    b11 = bias.reshape(1, 1)

    def wl_layout(f):
        # [p, dc*K + k] = w[k, dc*128 + p]
        return np.ascontiguousarray(
            f.reshape(K, DC, P).transpose(2, 1, 0).reshape(P, DC * K)
        )

    in_maps = []
    if mode == "bf16x1":
        w2 = filt.reshape(K, D)
        # bf16 stationary for d-chunks 0-2: [p, dc*K + k] = w[k, dc*128+p]
        wl = np.ascontiguousarray(
            w2[:, 0 : 3 * P].reshape(K, 3, P).transpose(2, 1, 0).reshape(P, 3 * K)
        ).astype(BF)
        # fp8 hi+lo stationary for chunk 3: [p, k] = w[k, 384+p]
        wc3 = np.ascontiguousarray(w2[:, 3 * P : D].T)  # [P, K] fp32
        w8h = wc3.astype(F8)
        w8l = (wc3 - w8h.astype(np.float32)).astype(F8)
        w8 = np.ascontiguousarray(np.concatenate([w8h, w8l], axis=1))
        ntile = S // TN
        sel = np.zeros((P, ntile), dtype=BF)
        for c in range(ntile):
            sel[c * K : (c + 1) * K, c] = 1
        zer16 = np.zeros((K, K), dtype=BF)
        b8 = np.broadcast_to(bias.reshape(1, 1), (8, 1)).astype(np.float32)
        b8 = np.ascontiguousarray(b8)
        for c in range(N_CORES):
            sl = slice(c * BC, (c + 1) * BC)
            xtf = embedded[sl].transpose(0, 2, 1)  # [BC, D, S] fp32 view
            xtc = np.ascontiguousarray(xtf[:, 0 : 3 * P]).astype(BF)
            xt8c = np.ascontiguousarray(xtf[:, 3 * P : D]).astype(F8)
            in_maps.append(
                {"xt": xtc, "xt8": xt8c, "w": wl, "w8": w8,
                 "bias": b8, "sel": sel, "zer": zer16}
            )
    elif mode == "bf16x3":
        wh = filt.astype(BF)
        wlo = (filt - wh.astype(np.float32)).astype(BF)
        whl = wl_layout(wh.astype(np.float32)).reshape(P, DC, K)
        wll = wl_layout(wlo.astype(np.float32)).reshape(P, DC, K)
        # per dc block: [wh (16) | zeros (16) | wl (16)]
        wcat = np.zeros((P, DC, 3 * K), dtype=np.float32)
        wcat[:, :, 0:K] = whl
        wcat[:, :, 2 * K : 3 * K] = wll
        wcat = wcat.reshape(P, DC * 3 * K).astype(BF)
        ones16 = np.ones((3 * K, 1), dtype=BF)
        zer16 = np.zeros((3 * K, K), dtype=BF)
        xh = embedded.astype(BF)
        xl = (embedded - xh.astype(np.float32)).astype(BF)
        for c in range(N_CORES):
            sl = slice(c * BC, (c + 1) * BC)
            xthc = np.ascontiguousarray(xh[sl].transpose(0, 2, 1))
            xtlc = np.ascontiguousarray(xl[sl].transpose(0, 2, 1))
            in_maps.append(
                {
                    "xth": xthc,
                    "xtl": xtlc,
                    "w": wcat,
                    "bias": b11,
                    "ones": ones16,
                    "zer": zer16,
                }
            )
    else:
        wl = wl_layout(filt)
        ones16 = np.ones((K, 1), dtype=np.float32)
        zer16 = np.zeros((K, K), dtype=np.float32)
        for c in range(N_CORES):
            xc = embedded[c * BC : (c + 1) * BC]
            xtc = np.ascontiguousarray(xc.transpose(0, 2, 1))
            in_maps.append(
                {"xt": xtc, "w": wl, "bias": b11, "ones": ones16, "zer": zer16}
            )
    return in_maps


def run(embedded, filt, bias, mode=DEFAULT_MODE, trace=False, **spmd_kwargs):
    nc = _get_nc(mode)
    in_maps = _prep_in_maps(embedded, filt, bias, mode)
    res = run_bass_kernel_spmd(
        nc, in_maps, list(range(N_CORES)), trace=trace, **spmd_kwargs
    )
    out = np.concatenate([res.results[c]["out"] for c in range(N_CORES)], axis=0)
    return out.astype(np.float32), res


def kernel(embedded, filt, bias):
    out, _ = run(embedded, filt, bias)
    return out



# revision 39
# speedup vs baseline: 1.0125x; 1.0125x over previous
"""Trainium2 Bass kernel for nn_CNNcond_9723805958518 (dense_cnn).

Computation (see reference.py): for embedded [B,S,D], filt [K*D,1], bias [1]:
    out[b, i] = sum_{k<K, d<D} embedded[b, i+k, d] * w[k, d] + bias
with K-1 zero frames padded past the end of the sequence
(B=32, S=4096, D=512, K=16).

Distribution: pure data parallelism over batch - 8 NeuronCores x 4 batches,
no collectives; each core gets its x slice pre-transposed to [D, S] on the
host so DMA loads are large contiguous reads (fp32/bf16 DMA-transpose of
this shape is not available on trn2). Measured ~139-147 us HW exec.

Per-core algorithm:
  Stage 1 (TensorE): Y[k, j] = sum_d x[j, d] * w[k, d] as matmuls with d on
    the contraction partitions: lhsT = w^T [128, 3*16] per 128-d chunk,
    rhs = x^T [128, 512 positions], accumulating 4 d-chunks in PSUM.
  Shift (DMA): out[i] needs sum_k Y[k, i+k] - a diagonal, which no compute
    engine can address (no per-partition column offsets). Y is written to a
    DRAM scratch with row pitch w and read back with stride w+1 per k-row,
    which lands Y[k, i+k] at [k, i]; row tails past S are pre-zeroed.
  Stage 2 (TensorE): column-sum of the 48 aligned rows (3 streams x 16 k)
    via a ones[48,1] matmul; bias is added on ScalarE during evacuation.

Precision ("bf16x3", default): x and w are split on the host into bf16
hi+lo pairs (same total bytes as fp32); stage 1 computes
xh*wh + xh*wl + xl*wh with fp32 PSUM accumulation (dropped xl*wl is ~2^-18
relative). The two xh passes share the moving operand, so one [128, 48]
stationary (wh | zeros | wl) computes both in a single 512-cycle matmul,
and the xl*wh pass accumulates onto the same PSUM tile's upper rows.
Y is evacuated as bf16 hi + lo + cross streams and stage 2 sums all three.
End-to-end ~6e-6 relative error - fp32-envelope class - at full PE rate
(plain fp32 matmul runs 4 cycles/row and would be the bottleneck at ~136us
PE per core; float32r is full-rate but tf32-rounds to ~1.6e-4 rel err).
Alternate modes kept for reference: "f32r", "f32" (build_nc_simple).

Scheduling notes are in build_nc_bf16x3's docstring. _split_multiwaits
works around this container's walrus build accepting only one sync-wait
command per instruction.
"""

import sys

import numpy as np

if "/opt/trn_rl_repo" not in sys.path:
    sys.path.append("/opt/trn_rl_repo")

import ml_dtypes

import concourse.bass as bass
import concourse.mybir as mybir
from concourse.bass_utils import run_bass_kernel_spmd
from concourse.tile import TileContext

# Problem constants (hardcoded per the harness contract).
B, S, D, K = 32, 4096, 512, 16
N_CORES = 8
BC = B // N_CORES  # batches per core
P = 128  # SBUF partitions / contraction size
DC = D // P  # d-chunks per position
TN = 512  # positions per matmul (PSUM bank = 512 fp32)
XH = 2048  # positions per x-tile load (SBUF budget)
NH = S // XH
NTH = XH // TN  # matmul tiles per x-tile
PITCH = S + K  # Y scratch row pitch
DIAG = PITCH + 1  # stride that walks the shifted diagonal
YFLAT = K * DIAG  # per-batch scratch elems (incl. rearrange pad)

_F32 = mybir.dt.float32
_BF16 = mybir.dt.bfloat16
_F8 = mybir.dt.float8e4
F8 = ml_dtypes.float8_e4m3
BF = ml_dtypes.bfloat16

DEFAULT_MODE = "bf16x1"


def _split_multiwaits(nc, max_waits=1):
    """This container's walrus build accepts at most one sync-wait command
    per instruction ("Too many sync wait commands" in setupSyncWait
    otherwise). Splitting a multi-wait instruction into a chain of
    same-engine single-wait Drains is semantically identical: waits are
    conjunctive and each engine executes its stream in order."""
    n = 0
    for fn in nc.m.functions:
        for blk in fn.blocks:
            out = []
            for ins in blk.instructions:
                si = getattr(ins, "sync_info", None)
                waits = list(si.on_wait) if si is not None and si.on_wait else []
                if len(waits) > max_waits:
                    extra = waits[: len(waits) - max_waits]
                    si.on_wait = waits[len(waits) - max_waits :]
                    for i in range(0, len(extra), max_waits):
                        # EVENT_SEMAPHORE is a pure wait carrier (~20-50 ns);
                        # a Drain here would flush the engine pipeline (on
                        # TensorE that costs microseconds per occurrence).
                        d = mybir.InstEventSemaphore(
                            name=nc.get_next_instruction_name(),
                            engine=ins.engine,
                            ins=[],
                            outs=[],
                            sync_info=mybir.SyncInfo(
                                on_wait=extra[i : i + max_waits], on_update=[]
                            ),
                        )
                        out.append(d)
                        n += 1
                out.append(ins)
            if len(out) != len(blk.instructions):
                blk.instructions = out
    return n


def build_nc_simple(mm_dt):
    """Single-pass variant: one x tensor / one w tensor of dtype mm_dt."""
    nc = bass.Bass("TRN2", debug=False)
    xt = nc.dram_tensor("xt", [BC, D, S], mm_dt, kind="ExternalInput")
    w = nc.dram_tensor("w", [P, DC * K], mm_dt, kind="ExternalInput")
    bias = nc.dram_tensor("bias", [1, 1], _F32, kind="ExternalInput")
    ones_d = nc.dram_tensor("ones", [K, 1], mm_dt, kind="ExternalInput")
    zer_d = nc.dram_tensor("zer", [K, K], mm_dt, kind="ExternalInput")
    out = nc.dram_tensor("out", [BC, S], _F32, kind="ExternalOutput")

    with TileContext(nc) as tc:
        with (
            tc.tile_pool(name="consts", bufs=1) as cpool,
            tc.tile_pool(name="xp", bufs=2) as xpool,
            tc.tile_pool(name="yp", bufs=2) as ypool,
            tc.tile_pool(name="afp", bufs=2) as apool,
            tc.tile_pool(name="obp", bufs=2) as opool,
            tc.tile_pool(name="psy", bufs=2, space="PSUM") as psy,
            tc.tile_pool(name="pso", bufs=2, space="PSUM") as pso,
            tc.tile_pool(name="dscr", bufs=1, space="DRAM") as dpool,
        ):
            wsb = cpool.tile([P, DC * K], mm_dt)
            nc.sync.dma_start(out=wsb[:, :], in_=w[:, :])
            bsb = cpool.tile([1, 1], _F32)
            nc.sync.dma_start(out=bsb[:, :], in_=bias[:, :])
            ones = cpool.tile([K, 1], mm_dt)
            nc.sync.dma_start(out=ones[:, :], in_=ones_d[:, :])
            zer = cpool.tile([K, K], mm_dt)
            nc.sync.dma_start(out=zer[:, :], in_=zer_d[:, :])
            yscr = dpool.tile([BC, YFLAT], mm_dt)

            for b in range(BC):
                tail = yscr[b, 0 : K * PITCH].rearrange("(k r) -> k r", r=PITCH)[
                    :, S:PITCH
                ]
                nc.sync.dma_start(out=tail, in_=zer[:, :])

            for b in range(BC):
                ybuf = ypool.tile([K, S], mm_dt)
                for h in range(NH):
                    xb = xpool.tile([P, DC * XH], mm_dt)
                    nc.sync.dma_start(
                        out=xb[:, :].rearrange("p (dc n) -> p dc n", n=XH),
                        in_=xt[b][:, h * XH : (h + 1) * XH].rearrange(
                            "(dc p) n -> p dc n", p=P
                        ),
                    )
                    for tt in range(NTH):
                        t = h * NTH + tt
                        py = psy.tile([K, TN], _F32)
                        for dc in range(DC):
                            nc.tensor.matmul(
                                py[:, :],
                                wsb[:, dc * K : (dc + 1) * K],
                                xb[:, dc * XH + tt * TN : dc * XH + (tt + 1) * TN],
                                start=(dc == 0),
                                stop=(dc == DC - 1),
                            )
                        nc.vector.tensor_copy(
                            ybuf[:, t * TN : (t + 1) * TN], py[:, :]
                        )

                ywr = yscr[b, 0 : K * PITCH].rearrange("(k r) -> k r", r=PITCH)[
                    :, 0:S
                ]
                nc.sync.dma_start(out=ywr, in_=ybuf[:, :])

                af = apool.tile([K, S], mm_dt)
                ard = yscr[b, :].rearrange("(k r) -> k r", r=DIAG)[:, 0:S]
                nc.sync.dma_start(out=af, in_=ard)

                ob = opool.tile([1, S], _F32)
                for t in range(S // TN):
                    po = pso.tile([1, TN], _F32)
                    nc.tensor.matmul(
                        po[:, :],
                        ones[:, :],
                        af[:, t * TN : (t + 1) * TN],
                        start=True,
                        stop=True,
                    )
                    nc.scalar.add(
                        ob[:, t * TN : (t + 1) * TN], po[:, :], bsb[0:1, 0:1]
                    )
                nc.sync.dma_start(out=out[b : b + 1, :], in_=ob[:, :])

    _split_multiwaits(nc)
    return nc


def build_nc_bf16x3(xh_=2048, xbufs=4):
    """3-pass bf16 split-precision variant (see module docstring).

    Pipelining details (from trace analysis of earlier versions):
      - x is loaded in 1 MB chunks; x-hi on the Sync HWDGE ring, x-lo on
        the Scalar ring; consts / scratch bounce / output go through SWDGE
        (gpsimd) so a waiting scratch DMA never head-of-line blocks the
        next x prefetch (HWDGE triggers are FIFO per ring). Batch 0 opens
        with two small chunks so the PE starts ~5 us earlier.
      - The two xh passes (xh*wh, xh*wl) share the moving operand, so one
        [128, 48] stationary (wh | zeros | wl - the zeros make the Yhl
        rows land 32-aligned) computes both in a single 512-cycle matmul;
        the xl*wh pass accumulates onto the Yhl rows directly.
      - The three Y streams (hi, lo, cross) live in one [96, S] SBUF tile
        at partition offsets 0/32/64, so each scratch bounce is ONE write
        + ONE read DMA: scratch rows are ordered (k, stream) with pitch
        w_, which makes the per-k diagonal shift a linear 3-D access
        pattern (strides 3*w_+1, w_, 1).
      - The scratch round trip has ~4-6 us latency and the PE queue is
        in-order, so stage 2 runs on two sub-ranges: the first is bounced
        after stage-1 tile 3 and consumed after tile 5; the second is
        bounced at batch end and consumed during the NEXT batch.
    """
    xh = xh_
    ntile = S // TN

    nc = bass.Bass("TRN2", debug=False)
    xth = nc.dram_tensor("xth", [BC, D, S], _BF16, kind="ExternalInput")
    xtl = nc.dram_tensor("xtl", [BC, D, S], _BF16, kind="ExternalInput")
    wd = nc.dram_tensor("w", [P, DC * 3 * K], _BF16, kind="ExternalInput")
    bias = nc.dram_tensor("bias", [1, 1], _F32, kind="ExternalInput")
    ones_d = nc.dram_tensor("ones", [3 * K, 1], _BF16, kind="ExternalInput")
    zer_d = nc.dram_tensor("zer", [3 * K, K], _BF16, kind="ExternalInput")
    out = nc.dram_tensor("out", [BC, S], _F32, kind="ExternalOutput")

    # Stage-2 sub-ranges (out columns) and the stage-1 tile after whose
    # evacuation each range's Y data (incl. K-1 lookahead) is complete.
    RANGES = [(0, 3 * TN), (3 * TN, S)]
    READY = [3, ntile - 1]
    G = 3  # streams

    with TileContext(nc) as tc:
        with (
            tc.tile_pool(name="consts", bufs=1) as cpool,
            tc.tile_pool(name="xph", bufs=xbufs) as xpool_h,
            tc.tile_pool(name="xpl", bufs=xbufs) as xpool_l,
            tc.tile_pool(name="ypool", bufs=2) as ypool,
            tc.tile_pool(name="afp", bufs=4) as apool,
            tc.tile_pool(name="obp", bufs=2) as opool,
            tc.tile_pool(name="psy", bufs=4, space="PSUM") as psy,
            tc.tile_pool(name="pso", bufs=3, space="PSUM") as pso,
            tc.tile_pool(name="dscr", bufs=1, space="DRAM") as dpool,
        ):
            wsb = cpool.tile([P, DC * 3 * K], _BF16)
            nc.gpsimd.dma_start(out=wsb[:, :], in_=wd[:, :])
            bsb = cpool.tile([1, 1], _F32)
            nc.gpsimd.dma_start(out=bsb[:, :], in_=bias[:, :])
            ones = cpool.tile([3 * K, 1], _BF16)
            nc.gpsimd.dma_start(out=ones[:, :], in_=ones_d[:, :])
            zer = cpool.tile([3 * K, K], _BF16)
            nc.gpsimd.dma_start(out=zer[:, :], in_=zer_d[:, :])

            # Scratch per (batch, range): rows ordered (k, stream), pitch
            # w_ = hi - lo + K; +K pad for the diagonal view.
            scr = {}
            for b in range(BC):
                for r, (lo, hi) in enumerate(RANGES):
                    w_ = hi - lo + K
                    scr[(b, r)] = dpool.tile(
                        [G * K * (w_ + 1)], _BF16, name=f"scr{r}_{b}"
                    )

            # Zero tails of the last-range scratches (reads past S).
            for b in range(BC):
                lo, hi = RANGES[-1]
                w_ = hi - lo + K
                s = scr[(b, len(RANGES) - 1)]
                v = s[:].rearrange("(g kr) -> g kr", g=G)[
                    :, 0 : K * w_
                ].rearrange("g (k r) -> g k r", r=w_)
                nc.gpsimd.dma_start(out=v[:, :, w_ - K : w_], in_=zer[:, :])

            def bounce(b, r, yb):
                """One write + one read DMA: Y[:, lo:wend) of all three
                streams to scratch rows (k, g), then the k-shifted
                diagonal back as af[48, hi-lo]."""
                lo, hi = RANGES[r]
                w_ = hi - lo + K
                wend = hi + K if r < len(RANGES) - 1 else S
                af = apool.tile([G * K, hi - lo], _BF16, name="af")
                s = scr[(b, r)]
                # g-blocks of pitch K*(w_+1); within a block, row k sits at
                # k*w_, and the k-shifted diagonal is the linear pattern
                # (k*(w_+1) + j) thanks to the +1 block padding... rows:
                # block g holds rows k at pitch w_, padded by K at the end.
                blk = s[:].rearrange("(g kr) -> g kr", g=G)
                rows = blk[:, 0 : K * w_].rearrange("g (k r) -> g k r", r=w_)
                for gi, yrow in enumerate((0, 32, 64)):
                    nc.gpsimd.dma_start(
                        out=rows[gi, :, 0 : wend - lo],
                        in_=yb[yrow : yrow + K, lo:wend],
                    )
                diag = blk[:, 0 : K * (w_ + 1)].rearrange(
                    "g (k r) -> g k r", r=w_ + 1
                )
                nc.gpsimd.dma_start(
                    out=af[:, :], in_=diag[:, :, 0 : hi - lo]
                )
                return af

            def stage2(ob, af, r):
                lo, hi = RANGES[r]
                for t2 in range(lo // TN, hi // TN):
                    po = pso.tile([1, TN], _F32, name="po")
                    j = t2 * TN - lo
                    nc.tensor.matmul(
                        po[:, :],
                        ones[:, :],
                        af[:, j : j + TN],
                        start=True,
                        stop=True,
                    )
                    nc.scalar.add(
                        ob[:, t2 * TN : (t2 + 1) * TN], po[:, :], bsb[0:1, 0:1]
                    )

            pending = None  # deferred stage-2 of the previous batch
            for b in range(BC):
                yb = ypool.tile([3 * 32, S], _BF16)
                ob = opool.tile([1, S], _F32)
                afs = {}
                # b=0 opens with two small chunks (faster first landing).
                if b == 0:
                    chunks = [(0, TN), (TN, TN)]
                    if xh > 2 * TN:
                        chunks.append((2 * TN, xh - 2 * TN))
                    chunks += [(i, xh) for i in range(xh, S, xh)]
                else:
                    chunks = [(i, xh) for i in range(0, S, xh)]
                for c0, cw in chunks:
                    xbh = xpool_h.tile([P, DC * xh], _BF16, name="xbh")
                    nc.sync.dma_start(
                        out=xbh[:, 0 : DC * cw].rearrange(
                            "p (dc n) -> p dc n", n=cw
                        ),
                        in_=xth[b][:, c0 : c0 + cw].rearrange(
                            "(dc p) n -> p dc n", p=P
                        ),
                    )
                    xbl = xpool_l.tile([P, DC * xh], _BF16, name="xbl")
                    nc.scalar.dma_start(
                        out=xbl[:, 0 : DC * cw].rearrange(
                            "p (dc n) -> p dc n", n=cw
                        ),
                        in_=xtl[b][:, c0 : c0 + cw].rearrange(
                            "(dc p) n -> p dc n", p=P
                        ),
                    )
                    for tt in range(cw // TN):
                        t = (c0 + tt * TN) // TN
                        if t == 2 and pending is not None:
                            pending()
                            pending = None
                        py48 = psy.tile([3 * K, TN], _F32, name="py48")
                        for dc in range(DC):
                            xsl = slice(
                                dc * cw + tt * TN, dc * cw + (tt + 1) * TN
                            )
                            nc.tensor.matmul(
                                py48[:, :],
                                wsb[:, dc * 3 * K : (dc + 1) * 3 * K],
                                xbh[:, xsl],
                                start=(dc == 0),
                                stop=False,
                            )
                        for dc in range(DC):
                            # xl*wh accumulates straight onto the Yhl rows
                            # (32-aligned PSUM slice), so no extra adds.
                            xsl = slice(
                                dc * cw + tt * TN, dc * cw + (tt + 1) * TN
                            )
                            nc.tensor.matmul(
                                py48[2 * K : 3 * K, :],
                                wsb[:, dc * 3 * K : dc * 3 * K + K],
                                xbl[:, xsl],
                                start=False,
                                stop=(dc == DC - 1),
                            )
                        # Evacuate: yh = bf16(Yhh), yl = bf16(Yhh - yh),
                        # yc = bf16(Yhl + Ylh), into one [96, S] tile at
                        # partition offsets 0 / 32 / 64.
                        yhs = yb[0:K, t * TN : (t + 1) * TN]
                        nc.vector.tensor_copy(yhs, py48[0:K, :])
                        nc.vector.tensor_tensor(
                            yb[32 : 32 + K, t * TN : (t + 1) * TN],
                            py48[0:K, :],
                            yhs,
                            mybir.AluOpType.subtract,
                        )
                        nc.vector.tensor_copy(
                            yb[64 : 64 + K, t * TN : (t + 1) * TN],
                            py48[2 * K : 3 * K, :],
                        )
                        for r, rdy in enumerate(READY):
                            if t == rdy:
                                afs[r] = bounce(b, r, yb)
                        if t == 5:
                            stage2(ob, afs[0], 0)

                def make_pending(b=b, afs=afs, ob=ob):
                    def emit():
                        stage2(ob, afs[1], 1)
                        nc.gpsimd.dma_start(out=out[b : b + 1, :], in_=ob[:, :])

                    return emit

                pending = make_pending()
            # Last batch: nothing left to hide behind; emit immediately.
            if pending is not None:
                pending()

    _split_multiwaits(nc)
    return nc


def build_nc_bf16x1(xh_=2048, xbufs=8):
    """Single-stream bf16 variant (~4e-3 rel err, well under the 2e-2 gate).

    Halves HBM traffic vs bf16x3: x is loaded once as bf16 (16.8 MB/core vs
    33.5 MB), which matters because the kernel is DMA-bound (360 GB/s/core,
    PE needs only ~29 us of the ~50 us DMA floor).

    Stage 2 uses a packed diagonal read: scratch rows are written at pitch
    PITCH1 = S+K and read at pitch PITCH1+1 (the k-shift), with each row's
    range [lo, lo+Cr*TN) split into Cr aligned TN-blocks stacked on
    partitions (c, k).  A [Cr*K, Cr] block-selector stationary
    sel[(c,k), o] = (c == o) then sums over k for Cr output blocks in ONE
    512-cycle matmul (4x fewer PE cycles than the ones-vector version).
    """
    xh = xh_
    ntile = S // TN
    PITCH1 = S + K

    nc = bass.Bass("TRN2", debug=False)
    xt = nc.dram_tensor("xt", [BC, 3 * P, S], _BF16, kind="ExternalInput")
    xt8 = nc.dram_tensor("xt8", [BC, P, S], _F8, kind="ExternalInput")
    wd = nc.dram_tensor("w", [P, 3 * K], _BF16, kind="ExternalInput")
    w8d = nc.dram_tensor("w8", [P, K], _BF16, kind="ExternalInput")
    bias = nc.dram_tensor("bias", [8, 1], _F32, kind="ExternalInput")
    sel_d = nc.dram_tensor("sel", [P, ntile], _BF16, kind="ExternalInput")
    zer_d = nc.dram_tensor("zer", [K, K], _BF16, kind="ExternalInput")
    out = nc.dram_tensor("out", [BC, S], _F32, kind="ExternalOutput")

    with TileContext(nc) as tc:
        with (
            tc.tile_pool(name="consts", bufs=1) as cpool,
            tc.tile_pool(name="xp", bufs=xbufs) as xpool,
            tc.tile_pool(name="x8p", bufs=3) as x8pool,
            tc.tile_pool(name="ypool", bufs=2) as ypool,
            tc.tile_pool(name="afp", bufs=4) as apool,
            tc.tile_pool(name="obp", bufs=4) as opool,
            tc.tile_pool(name="psy", bufs=6, space="PSUM") as psy,
            tc.tile_pool(name="pso", bufs=2, space="PSUM") as pso,
            tc.tile_pool(name="dscr", bufs=1, space="DRAM") as dpool,
        ):
            wsb = cpool.tile([P, 3 * K], _BF16)
            nc.gpsimd.dma_start(out=wsb[:, :], in_=wd[:, :])
            w8sb = cpool.tile([P, K], _BF16)
            nc.gpsimd.dma_start(out=w8sb[:, :], in_=w8d[:, :])
            bsb = cpool.tile([8, 1], _F32)
            nc.gpsimd.dma_start(out=bsb[:, :], in_=bias[:, :])
            selsb = cpool.tile([P, ntile], _BF16)
            nc.gpsimd.dma_start(out=selsb[:, :], in_=sel_d[:, :])
            zer = cpool.tile([K, K], _BF16)
            nc.gpsimd.dma_start(out=zer[:, :], in_=zer_d[:, :])

            scr = {}
            for b in range(BC):
                scr[b] = dpool.tile([K * (PITCH1 + 1)], _BF16, name=f"scr{b}")
                # Zero the [S, PITCH1) tail of each pitch-row once; the
                # shifted read of row k touches cols [S, S+k).
                wv = scr[b][0 : K * PITCH1].rearrange("(k r) -> k r", r=PITCH1)
                nc.gpsimd.dma_start(out=wv[:, S:PITCH1], in_=zer[:, :])

            def bounce_read(b, box, eng):
                """Whole-batch packed diagonal read: af[(c,k), j'] =
                Y[k, c*TN + j' + k], 8 blocks x 16 k = 128 partitions."""
                af = apool.tile([ntile * K, TN], _BF16, name="af")
                dv = scr[b][:].rearrange("(k r) -> k r", r=PITCH1 + 1)
                src = dv[:, 0 : ntile * TN].rearrange("k (c j) -> c k j", j=TN)
                eng.dma_start(out=af[:, :], in_=src)
                box["af"] = af

            def stage2(b, box):
                af = box["af"]
                po = pso.tile([ntile, TN], _F32, name="po")
                nc.tensor.matmul(
                    po[:, :], selsb[:, :], af[:, :], start=True, stop=True
                )
                ob = opool.tile([ntile, TN], _F32, name="ob")
                nc.scalar.add(ob[:, :], po[:, :], bsb[0:ntile, 0:1])
                # Output on the scalar HWDGE ring: the trigger directly
                # follows the bias-add on the same engine, so it never
                # waits (and SWDGE stays free for the mid-batch bounces).
                nc.scalar.dma_start(
                    out=out[b, :].rearrange("(c j) -> c j", j=TN),
                    in_=ob[:, :],
                )

            rings = [nc.sync, nc.scalar]
            ring_i = 0
            pending = None  # deferred bounce-read + stage-2 of previous batch
            for b in range(BC):
                yb = ypool.tile([K, S], _BF16)
                if b == 0:
                    # Ramp: small chunks so the PE starts early.
                    chunks = [(0, TN), (TN, TN), (2 * TN, 2 * TN)]
                    chunks += [(i, xh) for i in range(2 * TN * 2, S, xh)]
                elif b == BC - 1:
                    # Taper: small final chunks so the last tiles start
                    # right as the x stream ends (shorter drain).
                    chunks = [(0, xh), (xh, 2 * TN), (xh + 2 * TN, TN),
                              (xh + 3 * TN, TN)]
                else:
                    chunks = [(i, xh) for i in range(0, S, xh)]
                # Whole-batch fp8 load: one [128, S] DMA with 4 KB
                # contiguous runs per partition (the HWDGE queues are
                # descriptor-rate-bound, so 2 KB-run per-chunk fp8 loads
                # forfeit most of the byte savings).
                x8b = x8pool.tile([P, S], _F8, name="x8b")
                rings[ring_i % len(rings)].dma_start(
                    out=x8b[:, :], in_=xt8[b][:, :]
                )
                ring_i += 1
                for c0, cw in chunks:
                    ring = rings[ring_i % len(rings)]
                    xb = xpool.tile([P, 3 * xh], _BF16, name="xb")
                    ring.dma_start(
                        out=xb[:, 0 : 3 * cw].rearrange(
                            "p (dc n) -> p dc n", n=cw
                        ),
                        in_=xt[b][:, c0 : c0 + cw].rearrange(
                            "(dc p) n -> p dc n", p=P
                        ),
                    )
                    ring_i += 1
                    for tt in range(cw // TN):
                        t = (c0 + tt * TN) // TN
                        if t == 1 and pending is not None:
                            pending["read"](nc.gpsimd)
                        # The last batch consumes the previous batch's
                        # stage-2 at t==5, so its bias-add/out never sit
                        # behind the tail bounce triggers on the scalar
                        # engine; mid-run batches keep the roomier t==7.
                        tcons = 5 if b == BC - 1 else 7
                        if t == tcons and pending is not None:
                            pending["stage2"]()
                            pending = None
                        py = psy.tile([K, TN], _F32, name="py")
                        for dc in range(3):
                            xsl = slice(
                                dc * cw + tt * TN, dc * cw + (tt + 1) * TN
                            )
                            nc.tensor.matmul(
                                py[:, :],
                                wsb[:, dc * K : (dc + 1) * K],
                                xb[:, xsl],
                                start=(dc == 0),
                                stop=False,
                            )
                        # Chunk 3: mixed-dtype matmul (bf16 stationary
                        # x fp8 moving) - one matmul, same PE cost as the
                        # all-bf16 kernel, but the x stream drops from
                        # 4 MB to 3.5 MB per batch.
                        x8sl = slice(t * TN, (t + 1) * TN)  # batch-global
                        nc.tensor.matmul(
                            py[:, :],
                            w8sb[:, :],
                            x8b[:, x8sl],
                            start=False,
                            stop=True,
                        )
                        nc.vector.tensor_copy(
                            yb[:, t * TN : (t + 1) * TN], py[:, :]
                        )
                        wv = scr[b][0 : K * PITCH1].rearrange(
                            "(k r) -> k r", r=PITCH1
                        )
                        if b == BC - 1 and t == 6:
                            # Last batch: pre-write cols [0, 3088) on the
                            # now-idle scalar HWDGE ring so the post-t7
                            # write is tiny.
                            nc.scalar.dma_start(
                                out=wv[:, 0 : 6 * TN + K],
                                in_=yb[:, 0 : 6 * TN + K],
                            )
                        if t == ntile - 1:
                            if b == BC - 1:
                                nc.scalar.dma_start(
                                    out=wv[:, 6 * TN + K : S],
                                    in_=yb[:, 6 * TN + K : S],
                                )
                            else:
                                # Whole-batch scratch write on the (idle)
                                # SWDGE ring; consumed early next batch.
                                nc.gpsimd.dma_start(
                                    out=wv[:, 0:S], in_=yb[:, :]
                                )

                def make_pending(b=b):
                    box = {}
                    return {
                        "read": lambda eng: bounce_read(b, box, eng),
                        "stage2": lambda: stage2(b, box),
                    }

                pending = make_pending()
            if pending is not None:
                pending["read"](nc.sync)
                pending["stage2"]()

    _split_multiwaits(nc)
    return nc


_NC_CACHE = {}


def _get_nc(mode):
    if mode not in _NC_CACHE:
        if mode == "bf16x1":
            _NC_CACHE[mode] = build_nc_bf16x1()
        elif mode == "bf16x3":
            _NC_CACHE[mode] = build_nc_bf16x3()
        elif mode == "f32r":
            _NC_CACHE[mode] = build_nc_simple(mybir.dt.float32r)
        elif mode == "f32":
            _NC_CACHE[mode] = build_nc_simple(mybir.dt.float32)
        else:
            raise ValueError(mode)
    return _NC_CACHE[mode]


def _prep_in_maps(embedded, filt, bias, mode):
    embedded = np.ascontiguousarray(embedded, dtype=np.float32)
    filt = np.ascontiguousarray(filt, dtype=np.float32)
    bias = np.ascontiguousarray(bias, dtype=np.float32)
    b11 = bias.reshape(1, 1)

    def wl_layout(f):
        # [p, dc*K + k] = w[k, dc*128 + p]
        return np.ascontiguousarray(
            f.reshape(K, DC, P).transpose(2, 1, 0).reshape(P, DC * K)
        )

    in_maps = []
    if mode == "bf16x1":
        w2 = filt.reshape(K, D)
        # bf16 stationary for d-chunks 0-2: [p, dc*K + k] = w[k, dc*128+p]
        wl = np.ascontiguousarray(
            w2[:, 0 : 3 * P].reshape(K, 3, P).transpose(2, 1, 0).reshape(P, 3 * K)
        ).astype(BF)
        # fp8 hi+lo stationary for chunk 3: [p, k] = w[k, 384+p]
        wc3 = np.ascontiguousarray(w2[:, 3 * P : D].T)  # [P, K] fp32
        w8 = wc3.astype(BF)
        ntile = S // TN
        sel = np.zeros((P, ntile), dtype=BF)
        for c in range(ntile):
            sel[c * K : (c + 1) * K, c] = 1
        zer16 = np.zeros((K, K), dtype=BF)
        b8 = np.broadcast_to(bias.reshape(1, 1), (8, 1)).astype(np.float32)
        b8 = np.ascontiguousarray(b8)
        for c in range(N_CORES):
            sl = slice(c * BC, (c + 1) * BC)
            xtf = embedded[sl].transpose(0, 2, 1)  # [BC, D, S] fp32 view
            xtc = np.ascontiguousarray(xtf[:, 0 : 3 * P]).astype(BF)
            xt8c = np.ascontiguousarray(xtf[:, 3 * P : D]).astype(F8)
            in_maps.append(
                {"xt": xtc, "xt8": xt8c, "w": wl, "w8": w8,
                 "bias": b8, "sel": sel, "zer": zer16}
            )
    elif mode == "bf16x3":
        wh = filt.astype(BF)
        wlo = (filt - wh.astype(np.float32)).astype(BF)
        whl = wl_layout(wh.astype(np.float32)).reshape(P, DC, K)
        wll = wl_layout(wlo.astype(np.float32)).reshape(P, DC, K)
        # per dc block: [wh (16) | zeros (16) | wl (16)]
        wcat = np.zeros((P, DC, 3 * K), dtype=np.float32)
        wcat[:, :, 0:K] = whl
        wcat[:, :, 2 * K : 3 * K] = wll
        wcat = wcat.reshape(P, DC * 3 * K).astype(BF)
        ones16 = np.ones((3 * K, 1), dtype=BF)
        zer16 = np.zeros((3 * K, K), dtype=BF)
        xh = embedded.astype(BF)
        xl = (embedded - xh.astype(np.float32)).astype(BF)
        for c in range(N_CORES):
            sl = slice(c * BC, (c + 1) * BC)
            xthc = np.ascontiguousarray(xh[sl].transpose(0, 2, 1))
            xtlc = np.ascontiguousarray(xl[sl].transpose(0, 2, 1))
            in_maps.append(
                {
                    "xth": xthc,
                    "xtl": xtlc,
                    "w": wcat,
                    "bias": b11,
                    "ones": ones16,
                    "zer": zer16,
                }
            )
    else:
        wl = wl_layout(filt)
        ones16 = np.ones((K, 1), dtype=np.float32)
        zer16 = np.zeros((K, K), dtype=np.float32)
        for c in range(N_CORES):
            xc = embedded[c * BC : (c + 1) * BC]
            xtc = np.ascontiguousarray(xc.transpose(0, 2, 1))
            in_maps.append(
                {"xt": xtc, "w": wl, "bias": b11, "ones": ones16, "zer": zer16}
            )
    return in_maps


def run(embedded, filt, bias, mode=DEFAULT_MODE, trace=False, **spmd_kwargs):
    nc = _get_nc(mode)
    in_maps = _prep_in_maps(embedded, filt, bias, mode)
    res = run_bass_kernel_spmd(
        nc, in_maps, list(range(N_CORES)), trace=trace, **spmd_kwargs
    )
    out = np.concatenate([res.results[c]["out"] for c in range(N_CORES)], axis=0)
    return out.astype(np.float32), res


def kernel(embedded, filt, bias):
    out, _ = run(embedded, filt, bias)
    return out



# revision 40
# speedup vs baseline: 1.0217x; 1.0091x over previous
"""Trainium2 Bass kernel for nn_CNNcond_9723805958518 (dense_cnn).

Computation (see reference.py): for embedded [B,S,D], filt [K*D,1], bias [1]:
    out[b, i] = sum_{k<K, d<D} embedded[b, i+k, d] * w[k, d] + bias
with K-1 zero frames padded past the end of the sequence
(B=32, S=4096, D=512, K=16).

Distribution: pure data parallelism over batch - 8 NeuronCores x 4 batches,
no collectives; each core gets its x slice pre-transposed to [D, S] on the
host so DMA loads are large contiguous reads (fp32/bf16 DMA-transpose of
this shape is not available on trn2).

Default mode "bf16x1" (~3e-3 rel err vs the 2e-2 gate, ~70-75 us HW exec
vs ~145 us for the old bf16x3 default).  The kernel is HBM-bound: x as a
single bf16 stream is 16.8 MB/core, and the two HWDGE queues (sync +
scalar rings, ~185 GB/s each) run saturated at the ~370 GB/s per-core HBM
roofline for ~46 us; PE needs only ~33 us, and ~9 us runtime preamble +
~10 us drain make up the rest.  bf16 hi+lo ("bf16x3") doubles that stream
for precision nobody needs here.

Per-core algorithm (build_nc_bf16x1):
  Stage 1 (TensorE): Y[k, j] = sum_d x[j, d] * w[k, d]: per 512-position
    tile, 4 accumulating matmuls (one per 128-d chunk) with stationary
    w^T [128, 16]; DVE evacuates PSUM to a [16, S] bf16 yb tile.
  Shift (DMA): out[i] needs sum_k Y[k, i+k] - a diagonal no engine AP can
    address (no per-partition column offsets).  yb is written once per
    batch to a DRAM scratch at row pitch S+K (tails pre-zeroed) and read
    back at pitch S+K+1, PACKED: af[(c,k), j'] = Y[k, c*512 + j' + k] for
    the 8 TN-blocks c - 8 x 16 = 128 partitions in one DMA.
  Stage 2 (TensorE): one 512-cycle matmul with a [128, 8] block-selector
    stationary sel[(c,k), o] = (c == o) sums over k for all 8 output
    blocks at once; bias on ScalarE; out via the scalar HWDGE ring.

Scheduling (from perfetto/ntff iteration): x chunks alternate sync/scalar
HWDGE rings (2048-position chunks; batch 0 ramps 512/512/1024 so the PE
starts early, the last batch tapers ...512/512 so the final tiles start
as the stream ends).  Scratch write+read ride the otherwise-idle SWDGE
(gpsimd) ring mid-run; batch b's read fires at t==1 of b+1 and its
stage-2 at t==7 (t==5 for the last batch, so its bias/out never queue
behind the tail's bounce triggers on the Activation engine).  The last
batch pre-writes scratch cols [0, 3088) at t==6 on the then-idle scalar
ring, leaving only a tiny post-t7 write before the tail's read.

Measured traps (don't regress these): HWDGE x throughput is best with
4 KB descriptors - a host-side chunk-major layout with 16 KB descriptors
ran ~30% SLOWER; routing any x through SWDGE steals from the same HBM cap
and loses ~4 us; bias-add on the DVE head-of-line blocks the CAST evac
chain; machine-level run-to-run drift is +/-5-10 us, so only interleaved
A/B timing comparisons are valid (see ab.py).

Alternate modes kept for reference: "bf16x3", "f32r", "f32".
_split_multiwaits works around this container's walrus build accepting
only one sync-wait command per instruction.
"""

import sys

import numpy as np

if "/opt/trn_rl_repo" not in sys.path:
    sys.path.append("/opt/trn_rl_repo")

import ml_dtypes

import concourse.bass as bass
import concourse.mybir as mybir
from concourse.bass_utils import run_bass_kernel_spmd
from concourse.tile import TileContext

# Problem constants (hardcoded per the harness contract).
B, S, D, K = 32, 4096, 512, 16
N_CORES = 8
BC = B // N_CORES  # batches per core
P = 128  # SBUF partitions / contraction size
DC = D // P  # d-chunks per position
TN = 512  # positions per matmul (PSUM bank = 512 fp32)
XH = 2048  # positions per x-tile load (SBUF budget)
NH = S // XH
NTH = XH // TN  # matmul tiles per x-tile
PITCH = S + K  # Y scratch row pitch
DIAG = PITCH + 1  # stride that walks the shifted diagonal
YFLAT = K * DIAG  # per-batch scratch elems (incl. rearrange pad)

_F32 = mybir.dt.float32
_BF16 = mybir.dt.bfloat16
BF = ml_dtypes.bfloat16

DEFAULT_MODE = "bf16x1"


def _split_multiwaits(nc, max_waits=1):
    """This container's walrus build accepts at most one sync-wait command
    per instruction ("Too many sync wait commands" in setupSyncWait
    otherwise). Splitting a multi-wait instruction into a chain of
    same-engine single-wait Drains is semantically identical: waits are
    conjunctive and each engine executes its stream in order."""
    n = 0
    for fn in nc.m.functions:
        for blk in fn.blocks:
            out = []
            for ins in blk.instructions:
                si = getattr(ins, "sync_info", None)
                waits = list(si.on_wait) if si is not None and si.on_wait else []
                if len(waits) > max_waits:
                    extra = waits[: len(waits) - max_waits]
                    si.on_wait = waits[len(waits) - max_waits :]
                    for i in range(0, len(extra), max_waits):
                        # EVENT_SEMAPHORE is a pure wait carrier (~20-50 ns);
                        # a Drain here would flush the engine pipeline (on
                        # TensorE that costs microseconds per occurrence).
                        d = mybir.InstEventSemaphore(
                            name=nc.get_next_instruction_name(),
                            engine=ins.engine,
                            ins=[],
                            outs=[],
                            sync_info=mybir.SyncInfo(
                                on_wait=extra[i : i + max_waits], on_update=[]
                            ),
                        )
                        out.append(d)
                        n += 1
                out.append(ins)
            if len(out) != len(blk.instructions):
                blk.instructions = out
    return n


def build_nc_simple(mm_dt):
    """Single-pass variant: one x tensor / one w tensor of dtype mm_dt."""
    nc = bass.Bass("TRN2", debug=False)
    xt = nc.dram_tensor("xt", [BC, D, S], mm_dt, kind="ExternalInput")
    w = nc.dram_tensor("w", [P, DC * K], mm_dt, kind="ExternalInput")
    bias = nc.dram_tensor("bias", [1, 1], _F32, kind="ExternalInput")
    ones_d = nc.dram_tensor("ones", [K, 1], mm_dt, kind="ExternalInput")
    zer_d = nc.dram_tensor("zer", [K, K], mm_dt, kind="ExternalInput")
    out = nc.dram_tensor("out", [BC, S], _F32, kind="ExternalOutput")

    with TileContext(nc) as tc:
        with (
            tc.tile_pool(name="consts", bufs=1) as cpool,
            tc.tile_pool(name="xp", bufs=2) as xpool,
            tc.tile_pool(name="yp", bufs=2) as ypool,
            tc.tile_pool(name="afp", bufs=2) as apool,
            tc.tile_pool(name="obp", bufs=2) as opool,
            tc.tile_pool(name="psy", bufs=2, space="PSUM") as psy,
            tc.tile_pool(name="pso", bufs=2, space="PSUM") as pso,
            tc.tile_pool(name="dscr", bufs=1, space="DRAM") as dpool,
        ):
            wsb = cpool.tile([P, DC * K], mm_dt)
            nc.sync.dma_start(out=wsb[:, :], in_=w[:, :])
            bsb = cpool.tile([1, 1], _F32)
            nc.sync.dma_start(out=bsb[:, :], in_=bias[:, :])
            ones = cpool.tile([K, 1], mm_dt)
            nc.sync.dma_start(out=ones[:, :], in_=ones_d[:, :])
            zer = cpool.tile([K, K], mm_dt)
            nc.sync.dma_start(out=zer[:, :], in_=zer_d[:, :])
            yscr = dpool.tile([BC, YFLAT], mm_dt)

            for b in range(BC):
                tail = yscr[b, 0 : K * PITCH].rearrange("(k r) -> k r", r=PITCH)[
                    :, S:PITCH
                ]
                nc.sync.dma_start(out=tail, in_=zer[:, :])

            for b in range(BC):
                ybuf = ypool.tile([K, S], mm_dt)
                for h in range(NH):
                    xb = xpool.tile([P, DC * XH], mm_dt)
                    nc.sync.dma_start(
                        out=xb[:, :].rearrange("p (dc n) -> p dc n", n=XH),
                        in_=xt[b][:, h * XH : (h + 1) * XH].rearrange(
                            "(dc p) n -> p dc n", p=P
                        ),
                    )
                    for tt in range(NTH):
                        t = h * NTH + tt
                        py = psy.tile([K, TN], _F32)
                        for dc in range(DC):
                            nc.tensor.matmul(
                                py[:, :],
                                wsb[:, dc * K : (dc + 1) * K],
                                xb[:, dc * XH + tt * TN : dc * XH + (tt + 1) * TN],
                                start=(dc == 0),
                                stop=(dc == DC - 1),
                            )
                        nc.vector.tensor_copy(
                            ybuf[:, t * TN : (t + 1) * TN], py[:, :]
                        )

                ywr = yscr[b, 0 : K * PITCH].rearrange("(k r) -> k r", r=PITCH)[
                    :, 0:S
                ]
                nc.sync.dma_start(out=ywr, in_=ybuf[:, :])

                af = apool.tile([K, S], mm_dt)
                ard = yscr[b, :].rearrange("(k r) -> k r", r=DIAG)[:, 0:S]
                nc.sync.dma_start(out=af, in_=ard)

                ob = opool.tile([1, S], _F32)
                for t in range(S // TN):
                    po = pso.tile([1, TN], _F32)
                    nc.tensor.matmul(
                        po[:, :],
                        ones[:, :],
                        af[:, t * TN : (t + 1) * TN],
                        start=True,
                        stop=True,
                    )
                    nc.scalar.add(
                        ob[:, t * TN : (t + 1) * TN], po[:, :], bsb[0:1, 0:1]
                    )
                nc.sync.dma_start(out=out[b : b + 1, :], in_=ob[:, :])

    _split_multiwaits(nc)
    return nc


def build_nc_bf16x3(xh_=2048, xbufs=4):
    """3-pass bf16 split-precision variant (see module docstring).

    Pipelining details (from trace analysis of earlier versions):
      - x is loaded in 1 MB chunks; x-hi on the Sync HWDGE ring, x-lo on
        the Scalar ring; consts / scratch bounce / output go through SWDGE
        (gpsimd) so a waiting scratch DMA never head-of-line blocks the
        next x prefetch (HWDGE triggers are FIFO per ring). Batch 0 opens
        with two small chunks so the PE starts ~5 us earlier.
      - The two xh passes (xh*wh, xh*wl) share the moving operand, so one
        [128, 48] stationary (wh | zeros | wl - the zeros make the Yhl
        rows land 32-aligned) computes both in a single 512-cycle matmul;
        the xl*wh pass accumulates onto the Yhl rows directly.
      - The three Y streams (hi, lo, cross) live in one [96, S] SBUF tile
        at partition offsets 0/32/64, so each scratch bounce is ONE write
        + ONE read DMA: scratch rows are ordered (k, stream) with pitch
        w_, which makes the per-k diagonal shift a linear 3-D access
        pattern (strides 3*w_+1, w_, 1).
      - The scratch round trip has ~4-6 us latency and the PE queue is
        in-order, so stage 2 runs on two sub-ranges: the first is bounced
        after stage-1 tile 3 and consumed after tile 5; the second is
        bounced at batch end and consumed during the NEXT batch.
    """
    xh = xh_
    ntile = S // TN

    nc = bass.Bass("TRN2", debug=False)
    xth = nc.dram_tensor("xth", [BC, D, S], _BF16, kind="ExternalInput")
    xtl = nc.dram_tensor("xtl", [BC, D, S], _BF16, kind="ExternalInput")
    wd = nc.dram_tensor("w", [P, DC * 3 * K], _BF16, kind="ExternalInput")
    bias = nc.dram_tensor("bias", [1, 1], _F32, kind="ExternalInput")
    ones_d = nc.dram_tensor("ones", [3 * K, 1], _BF16, kind="ExternalInput")
    zer_d = nc.dram_tensor("zer", [3 * K, K], _BF16, kind="ExternalInput")
    out = nc.dram_tensor("out", [BC, S], _F32, kind="ExternalOutput")

    # Stage-2 sub-ranges (out columns) and the stage-1 tile after whose
    # evacuation each range's Y data (incl. K-1 lookahead) is complete.
    RANGES = [(0, 3 * TN), (3 * TN, S)]
    READY = [3, ntile - 1]
    G = 3  # streams

    with TileContext(nc) as tc:
        with (
            tc.tile_pool(name="consts", bufs=1) as cpool,
            tc.tile_pool(name="xph", bufs=xbufs) as xpool_h,
            tc.tile_pool(name="xpl", bufs=xbufs) as xpool_l,
            tc.tile_pool(name="ypool", bufs=2) as ypool,
            tc.tile_pool(name="afp", bufs=4) as apool,
            tc.tile_pool(name="obp", bufs=2) as opool,
            tc.tile_pool(name="psy", bufs=4, space="PSUM") as psy,
            tc.tile_pool(name="pso", bufs=3, space="PSUM") as pso,
            tc.tile_pool(name="dscr", bufs=1, space="DRAM") as dpool,
        ):
            wsb = cpool.tile([P, DC * 3 * K], _BF16)
            nc.gpsimd.dma_start(out=wsb[:, :], in_=wd[:, :])
            bsb = cpool.tile([1, 1], _F32)
            nc.gpsimd.dma_start(out=bsb[:, :], in_=bias[:, :])
            ones = cpool.tile([3 * K, 1], _BF16)
            nc.gpsimd.dma_start(out=ones[:, :], in_=ones_d[:, :])
            zer = cpool.tile([3 * K, K], _BF16)
            nc.gpsimd.dma_start(out=zer[:, :], in_=zer_d[:, :])

            # Scratch per (batch, range): rows ordered (k, stream), pitch
            # w_ = hi - lo + K; +K pad for the diagonal view.
            scr = {}
            for b in range(BC):
                for r, (lo, hi) in enumerate(RANGES):
                    w_ = hi - lo + K
                    scr[(b, r)] = dpool.tile(
                        [G * K * (w_ + 1)], _BF16, name=f"scr{r}_{b}"
                    )

            # Zero tails of the last-range scratches (reads past S).
            for b in range(BC):
                lo, hi = RANGES[-1]
                w_ = hi - lo + K
                s = scr[(b, len(RANGES) - 1)]
                v = s[:].rearrange("(g kr) -> g kr", g=G)[
                    :, 0 : K * w_
                ].rearrange("g (k r) -> g k r", r=w_)
                nc.gpsimd.dma_start(out=v[:, :, w_ - K : w_], in_=zer[:, :])

            def bounce(b, r, yb):
                """One write + one read DMA: Y[:, lo:wend) of all three
                streams to scratch rows (k, g), then the k-shifted
                diagonal back as af[48, hi-lo]."""
                lo, hi = RANGES[r]
                w_ = hi - lo + K
                wend = hi + K if r < len(RANGES) - 1 else S
                af = apool.tile([G * K, hi - lo], _BF16, name="af")
                s = scr[(b, r)]
                # g-blocks of pitch K*(w_+1); within a block, row k sits at
                # k*w_, and the k-shifted diagonal is the linear pattern
                # (k*(w_+1) + j) thanks to the +1 block padding... rows:
                # block g holds rows k at pitch w_, padded by K at the end.
                blk = s[:].rearrange("(g kr) -> g kr", g=G)
                rows = blk[:, 0 : K * w_].rearrange("g (k r) -> g k r", r=w_)
                for gi, yrow in enumerate((0, 32, 64)):
                    nc.gpsimd.dma_start(
                        out=rows[gi, :, 0 : wend - lo],
                        in_=yb[yrow : yrow + K, lo:wend],
                    )
                diag = blk[:, 0 : K * (w_ + 1)].rearrange(
                    "g (k r) -> g k r", r=w_ + 1
                )
                nc.gpsimd.dma_start(
                    out=af[:, :], in_=diag[:, :, 0 : hi - lo]
                )
                return af

            def stage2(ob, af, r):
                lo, hi = RANGES[r]
                for t2 in range(lo // TN, hi // TN):
                    po = pso.tile([1, TN], _F32, name="po")
                    j = t2 * TN - lo
                    nc.tensor.matmul(
                        po[:, :],
                        ones[:, :],
                        af[:, j : j + TN],
                        start=True,
                        stop=True,
                    )
                    nc.scalar.add(
                        ob[:, t2 * TN : (t2 + 1) * TN], po[:, :], bsb[0:1, 0:1]
                    )

            pending = None  # deferred stage-2 of the previous batch
            for b in range(BC):
                yb = ypool.tile([3 * 32, S], _BF16)
                ob = opool.tile([1, S], _F32)
                afs = {}
                # b=0 opens with two small chunks (faster first landing).
                if b == 0:
                    chunks = [(0, TN), (TN, TN)]
                    if xh > 2 * TN:
                        chunks.append((2 * TN, xh - 2 * TN))
                    chunks += [(i, xh) for i in range(xh, S, xh)]
                else:
                    chunks = [(i, xh) for i in range(0, S, xh)]
                for c0, cw in chunks:
                    xbh = xpool_h.tile([P, DC * xh], _BF16, name="xbh")
                    nc.sync.dma_start(
                        out=xbh[:, 0 : DC * cw].rearrange(
                            "p (dc n) -> p dc n", n=cw
                        ),
                        in_=xth[b][:, c0 : c0 + cw].rearrange(
                            "(dc p) n -> p dc n", p=P
                        ),
                    )
                    xbl = xpool_l.tile([P, DC * xh], _BF16, name="xbl")
                    nc.scalar.dma_start(
                        out=xbl[:, 0 : DC * cw].rearrange(
                            "p (dc n) -> p dc n", n=cw
                        ),
                        in_=xtl[b][:, c0 : c0 + cw].rearrange(
                            "(dc p) n -> p dc n", p=P
                        ),
                    )
                    for tt in range(cw // TN):
                        t = (c0 + tt * TN) // TN
                        if t == 2 and pending is not None:
                            pending()
                            pending = None
                        py48 = psy.tile([3 * K, TN], _F32, name="py48")
                        for dc in range(DC):
                            xsl = slice(
                                dc * cw + tt * TN, dc * cw + (tt + 1) * TN
                            )
                            nc.tensor.matmul(
                                py48[:, :],
                                wsb[:, dc * 3 * K : (dc + 1) * 3 * K],
                                xbh[:, xsl],
                                start=(dc == 0),
                                stop=False,
                            )
                        for dc in range(DC):
                            # xl*wh accumulates straight onto the Yhl rows
                            # (32-aligned PSUM slice), so no extra adds.
                            xsl = slice(
                                dc * cw + tt * TN, dc * cw + (tt + 1) * TN
                            )
                            nc.tensor.matmul(
                                py48[2 * K : 3 * K, :],
                                wsb[:, dc * 3 * K : dc * 3 * K + K],
                                xbl[:, xsl],
                                start=False,
                                stop=(dc == DC - 1),
                            )
                        # Evacuate: yh = bf16(Yhh), yl = bf16(Yhh - yh),
                        # yc = bf16(Yhl + Ylh), into one [96, S] tile at
                        # partition offsets 0 / 32 / 64.
                        yhs = yb[0:K, t * TN : (t + 1) * TN]
                        nc.vector.tensor_copy(yhs, py48[0:K, :])
                        nc.vector.tensor_tensor(
                            yb[32 : 32 + K, t * TN : (t + 1) * TN],
                            py48[0:K, :],
                            yhs,
                            mybir.AluOpType.subtract,
                        )
                        nc.vector.tensor_copy(
                            yb[64 : 64 + K, t * TN : (t + 1) * TN],
                            py48[2 * K : 3 * K, :],
                        )
                        for r, rdy in enumerate(READY):
                            if t == rdy:
                                afs[r] = bounce(b, r, yb)
                        if t == 5:
                            stage2(ob, afs[0], 0)

                def make_pending(b=b, afs=afs, ob=ob):
                    def emit():
                        stage2(ob, afs[1], 1)
                        nc.gpsimd.dma_start(out=out[b : b + 1, :], in_=ob[:, :])

                    return emit

                pending = make_pending()
            # Last batch: nothing left to hide behind; emit immediately.
            if pending is not None:
                pending()

    _split_multiwaits(nc)
    return nc


def build_nc_bf16x1(xh_=2048, xbufs=8):
    """Single-stream bf16 variant (~4e-3 rel err, well under the 2e-2 gate).

    Halves HBM traffic vs bf16x3: x is loaded once as bf16 (16.8 MB/core vs
    33.5 MB), which matters because the kernel is DMA-bound (360 GB/s/core,
    PE needs only ~29 us of the ~50 us DMA floor).

    Stage 2 uses a packed diagonal read: scratch rows are written at pitch
    PITCH1 = S+K and read at pitch PITCH1+1 (the k-shift), with each row's
    range [lo, lo+Cr*TN) split into Cr aligned TN-blocks stacked on
    partitions (c, k).  A [Cr*K, Cr] block-selector stationary
    sel[(c,k), o] = (c == o) then sums over k for Cr output blocks in ONE
    512-cycle matmul (4x fewer PE cycles than the ones-vector version).
    """
    xh = xh_
    ntile = S // TN
    PITCH1 = S + K

    nc = bass.Bass("TRN2", debug=False)
    xt = nc.dram_tensor("xt", [BC, D, S], _BF16, kind="ExternalInput")
    wd = nc.dram_tensor("w", [P, DC * K], _BF16, kind="ExternalInput")
    bias = nc.dram_tensor("bias", [8, 1], _F32, kind="ExternalInput")
    sel_d = nc.dram_tensor("sel", [P, ntile], _BF16, kind="ExternalInput")
    zer_d = nc.dram_tensor("zer", [K, K], _BF16, kind="ExternalInput")
    out = nc.dram_tensor("out", [BC, S], _F32, kind="ExternalOutput")

    with TileContext(nc) as tc:
        with (
            tc.tile_pool(name="consts", bufs=1) as cpool,
            tc.tile_pool(name="xp", bufs=xbufs) as xpool,
            tc.tile_pool(name="ypool", bufs=2) as ypool,
            tc.tile_pool(name="afp", bufs=4) as apool,
            tc.tile_pool(name="obp", bufs=4) as opool,
            tc.tile_pool(name="psy", bufs=6, space="PSUM") as psy,
            tc.tile_pool(name="pso", bufs=2, space="PSUM") as pso,
            tc.tile_pool(name="dscr", bufs=1, space="DRAM") as dpool,
        ):
            wsb = cpool.tile([P, DC * K], _BF16)
            nc.gpsimd.dma_start(out=wsb[:, :], in_=wd[:, :])
            bsb = cpool.tile([8, 1], _F32)
            nc.gpsimd.dma_start(out=bsb[:, :], in_=bias[:, :])
            selsb = cpool.tile([P, ntile], _BF16)
            nc.gpsimd.dma_start(out=selsb[:, :], in_=sel_d[:, :])
            zer = cpool.tile([K, K], _BF16)
            nc.gpsimd.dma_start(out=zer[:, :], in_=zer_d[:, :])

            scr = {}
            for b in range(BC):
                scr[b] = dpool.tile([K * (PITCH1 + 1)], _BF16, name=f"scr{b}")
                # Zero the [S, PITCH1) tail of each pitch-row once; the
                # shifted read of row k touches cols [S, S+k).
                wv = scr[b][0 : K * PITCH1].rearrange("(k r) -> k r", r=PITCH1)
                nc.gpsimd.dma_start(out=wv[:, S:PITCH1], in_=zer[:, :])

            def bounce_read(b, box, eng):
                """Whole-batch packed diagonal read: af[(c,k), j'] =
                Y[k, c*TN + j' + k], 8 blocks x 16 k = 128 partitions."""
                af = apool.tile([ntile * K, TN], _BF16, name="af")
                dv = scr[b][:].rearrange("(k r) -> k r", r=PITCH1 + 1)
                src = dv[:, 0 : ntile * TN].rearrange("k (c j) -> c k j", j=TN)
                eng.dma_start(out=af[:, :], in_=src)
                box["af"] = af

            def stage2(b, box):
                af = box["af"]
                po = pso.tile([ntile, TN], _F32, name="po")
                nc.tensor.matmul(
                    po[:, :], selsb[:, :], af[:, :], start=True, stop=True
                )
                ob = opool.tile([ntile, TN], _F32, name="ob")
                nc.scalar.add(ob[:, :], po[:, :], bsb[0:ntile, 0:1])
                # Output on the scalar HWDGE ring: the trigger directly
                # follows the bias-add on the same engine, so it never
                # waits (and SWDGE stays free for the mid-batch bounces).
                nc.scalar.dma_start(
                    out=out[b, :].rearrange("(c j) -> c j", j=TN),
                    in_=ob[:, :],
                )

            rings = [nc.sync, nc.scalar]
            ring_i = 0
            pending = None  # deferred bounce-read + stage-2 of previous batch
            for b in range(BC):
                yb = ypool.tile([K, S], _BF16)
                if b == 0:
                    # Ramp: small chunks so the PE starts early.
                    chunks = [(0, TN), (TN, TN), (2 * TN, 2 * TN)]
                    chunks += [(i, xh) for i in range(2 * TN * 2, S, xh)]
                elif b == BC - 1:
                    # Taper: small final chunks so the last tiles start
                    # right as the x stream ends (shorter drain).
                    chunks = [(0, xh), (xh, 2 * TN), (xh + 2 * TN, TN),
                              (xh + 3 * TN, TN)]
                else:
                    chunks = [(i, xh) for i in range(0, S, xh)]
                for c0, cw in chunks:
                    xb = xpool.tile([P, DC * xh], _BF16, name="xb")
                    rings[ring_i % len(rings)].dma_start(
                        out=xb[:, 0 : DC * cw].rearrange(
                            "p (dc n) -> p dc n", n=cw
                        ),
                        in_=xt[b][:, c0 : c0 + cw].rearrange(
                            "(dc p) n -> p dc n", p=P
                        ),
                    )
                    ring_i += 1
                    for tt in range(cw // TN):
                        t = (c0 + tt * TN) // TN
                        if t == 1 and pending is not None:
                            pending["read"](nc.gpsimd)
                        # The last batch consumes the previous batch's
                        # stage-2 at t==5, so its bias-add/out never sit
                        # behind the tail bounce triggers on the scalar
                        # engine; mid-run batches keep the roomier t==7.
                        tcons = 5 if b == BC - 1 else 7
                        if t == tcons and pending is not None:
                            pending["stage2"]()
                            pending = None
                        py = psy.tile([K, TN], _F32, name="py")
                        for dc in range(DC):
                            xsl = slice(
                                dc * cw + tt * TN, dc * cw + (tt + 1) * TN
                            )
                            nc.tensor.matmul(
                                py[:, :],
                                wsb[:, dc * K : (dc + 1) * K],
                                xb[:, xsl],
                                start=(dc == 0),
                                stop=(dc == DC - 1),
                            )
                        nc.vector.tensor_copy(
                            yb[:, t * TN : (t + 1) * TN], py[:, :]
                        )
                        wv = scr[b][0 : K * PITCH1].rearrange(
                            "(k r) -> k r", r=PITCH1
                        )
                        if b == BC - 1 and t == 6:
                            # Last batch: pre-write cols [0, 3088) on the
                            # now-idle scalar HWDGE ring so the post-t7
                            # write is tiny.
                            nc.scalar.dma_start(
                                out=wv[:, 0 : 6 * TN + K],
                                in_=yb[:, 0 : 6 * TN + K],
                            )
                        if t == ntile - 1:
                            if b == BC - 1:
                                nc.scalar.dma_start(
                                    out=wv[:, 6 * TN + K : S],
                                    in_=yb[:, 6 * TN + K : S],
                                )
                            else:
                                # Whole-batch scratch write on the (idle)
                                # SWDGE ring; consumed early next batch.
                                nc.gpsimd.dma_start(
                                    out=wv[:, 0:S], in_=yb[:, :]
                                )

                def make_pending(b=b):
                    box = {}
                    return {
                        "read": lambda eng: bounce_read(b, box, eng),
                        "stage2": lambda: stage2(b, box),
                    }

                pending = make_pending()
            if pending is not None:
                pending["read"](nc.sync)
                pending["stage2"]()

    _split_multiwaits(nc)
    return nc


_NC_CACHE = {}


def _get_nc(mode):
    if mode not in _NC_CACHE:
        if mode == "bf16x1":
            _NC_CACHE[mode] = build_nc_bf16x1()
        elif mode == "bf16x3":
            _NC_CACHE[mode] = build_nc_bf16x3()
        elif mode == "f32r":
            _NC_CACHE[mode] = build_nc_simple(mybir.dt.float32r)
        elif mode == "f32":
            _NC_CACHE[mode] = build_nc_simple(mybir.dt.float32)
        else:
            raise ValueError(mode)
    return _NC_CACHE[mode]


def _prep_in_maps(embedded, filt, bias, mode):
    embedded = np.ascontiguousarray(embedded, dtype=np.float32)
    filt = np.ascontiguousarray(filt, dtype=np.float32)
    bias = np.ascontiguousarray(bias, dtype=np.float32)
    b11 = bias.reshape(1, 1)

    def wl_layout(f):
        # [p, dc*K + k] = w[k, dc*128 + p]
        return np.ascontiguousarray(
            f.reshape(K, DC, P).transpose(2, 1, 0).reshape(P, DC * K)
        )

    in_maps = []
    if mode == "bf16x1":
        wl = wl_layout(filt.astype(BF).astype(np.float32)).astype(BF)
        ntile = S // TN
        sel = np.zeros((P, ntile), dtype=BF)
        for c in range(ntile):
            sel[c * K : (c + 1) * K, c] = 1
        zer16 = np.zeros((K, K), dtype=BF)
        b8 = np.broadcast_to(bias.reshape(1, 1), (8, 1)).astype(np.float32)
        b8 = np.ascontiguousarray(b8)
        xh = embedded.astype(BF)
        for c in range(N_CORES):
            sl = slice(c * BC, (c + 1) * BC)
            xtc = np.ascontiguousarray(xh[sl].transpose(0, 2, 1))
            in_maps.append(
                {"xt": xtc, "w": wl, "bias": b8, "sel": sel, "zer": zer16}
            )
    elif mode == "bf16x3":
        wh = filt.astype(BF)
        wlo = (filt - wh.astype(np.float32)).astype(BF)
        whl = wl_layout(wh.astype(np.float32)).reshape(P, DC, K)
        wll = wl_layout(wlo.astype(np.float32)).reshape(P, DC, K)
        # per dc block: [wh (16) | zeros (16) | wl (16)]
        wcat = np.zeros((P, DC, 3 * K), dtype=np.float32)
        wcat[:, :, 0:K] = whl
        wcat[:, :, 2 * K : 3 * K] = wll
        wcat = wcat.reshape(P, DC * 3 * K).astype(BF)
        ones16 = np.ones((3 * K, 1), dtype=BF)
        zer16 = np.zeros((3 * K, K), dtype=BF)
        xh = embedded.astype(BF)
        xl = (embedded - xh.astype(np.float32)).astype(BF)
        for c in range(N_CORES):
            sl = slice(c * BC, (c + 1) * BC)
            xthc = np.ascontiguousarray(xh[sl].transpose(0, 2, 1))
            xtlc = np.ascontiguousarray(xl[sl].transpose(0, 2, 1))
            in_maps.append(
                {
                    "xth": xthc,
                    "xtl": xtlc,
                    "w": wcat,
                    "bias": b11,
                    "ones": ones16,
                    "zer": zer16,
                }
            )
    else:
        wl = wl_layout(filt)
        ones16 = np.ones((K, 1), dtype=np.float32)
        zer16 = np.zeros((K, K), dtype=np.float32)
        for c in range(N_CORES):
            xc = embedded[c * BC : (c + 1) * BC]
            xtc = np.ascontiguousarray(xc.transpose(0, 2, 1))
            in_maps.append(
                {"xt": xtc, "w": wl, "bias": b11, "ones": ones16, "zer": zer16}
            )
    return in_maps


def run(embedded, filt, bias, mode=DEFAULT_MODE, trace=False, **spmd_kwargs):
    nc = _get_nc(mode)
    in_maps = _prep_in_maps(embedded, filt, bias, mode)
    res = run_bass_kernel_spmd(
        nc, in_maps, list(range(N_CORES)), trace=trace, **spmd_kwargs
    )
    out = np.concatenate([res.results[c]["out"] for c in range(N_CORES)], axis=0)
    return out.astype(np.float32), res


def kernel(embedded, filt, bias):
    out, _ = run(embedded, filt, bias)
    return out



# revision 41
# speedup vs baseline: 1.0586x; 1.0362x over previous
"""Trainium2 Bass kernel for nn_CNNcond_9723805958518 (dense_cnn).

Computation (see reference.py): for embedded [B,S,D], filt [K*D,1], bias [1]:
    out[b, i] = sum_{k<K, d<D} embedded[b, i+k, d] * w[k, d] + bias
with K-1 zero frames padded past the end of the sequence
(B=32, S=4096, D=512, K=16).

Distribution: pure data parallelism over batch - 8 NeuronCores x 4 batches,
no collectives; each core gets its x slice pre-transposed to [D, S] on the
host so DMA loads are large contiguous reads (fp32/bf16 DMA-transpose of
this shape is not available on trn2).

Default mode "bf16x1" (~3e-3 rel err vs the 2e-2 gate, ~70-75 us HW exec
vs ~145 us for the old bf16x3 default).  The kernel is HBM-bound: x as a
single bf16 stream is 16.8 MB/core, and the two HWDGE queues (sync +
scalar rings, ~185 GB/s each) run saturated at the ~370 GB/s per-core HBM
roofline for ~46 us; PE needs only ~33 us, and ~9 us runtime preamble +
~10 us drain make up the rest.  bf16 hi+lo ("bf16x3") doubles that stream
for precision nobody needs here.

Per-core algorithm (build_nc_bf16x1):
  Stage 1 (TensorE): Y[k, j] = sum_d x[j, d] * w[k, d]: per 512-position
    tile, 4 accumulating matmuls (one per 128-d chunk) with stationary
    w^T [128, 16]; DVE evacuates PSUM to a [16, S] bf16 yb tile.
  Shift (DMA): out[i] needs sum_k Y[k, i+k] - a diagonal no engine AP can
    address (no per-partition column offsets).  yb is written once per
    batch to a DRAM scratch at row pitch S+K (tails pre-zeroed) and read
    back at pitch S+K+1, PACKED: af[(c,k), j'] = Y[k, c*512 + j' + k] for
    the 8 TN-blocks c - 8 x 16 = 128 partitions in one DMA.
  Stage 2 (TensorE): one 512-cycle matmul with a [128, 8] block-selector
    stationary sel[(c,k), o] = (c == o) sums over k for all 8 output
    blocks at once; bias on ScalarE; out via the scalar HWDGE ring.

Scheduling (from perfetto/ntff iteration): x chunks alternate sync/scalar
HWDGE rings (2048-position chunks; batch 0 ramps 512/512/1024 so the PE
starts early, the last batch tapers ...512/512 so the final tiles start
as the stream ends).  Scratch write+read ride the otherwise-idle SWDGE
(gpsimd) ring mid-run; batch b's read fires at t==1 of b+1 and its
stage-2 at t==7 (t==5 for the last batch, so its bias/out never queue
behind the tail's bounce triggers on the Activation engine).  The last
batch pre-writes scratch cols [0, 3088) at t==6 on the then-idle scalar
ring, leaving only a tiny post-t7 write before the tail's read.

Measured traps (don't regress these): HWDGE x throughput is best with
4 KB descriptors - a host-side chunk-major layout with 16 KB descriptors
ran ~30% SLOWER; routing any x through SWDGE steals from the same HBM cap
and loses ~4 us; bias-add on the DVE head-of-line blocks the CAST evac
chain; machine-level run-to-run drift is +/-5-10 us, so only interleaved
A/B timing comparisons are valid (see ab.py).

Alternate modes kept for reference: "bf16x3", "f32r", "f32".
_split_multiwaits works around this container's walrus build accepting
only one sync-wait command per instruction.
"""

import sys

import numpy as np

if "/opt/trn_rl_repo" not in sys.path:
    sys.path.append("/opt/trn_rl_repo")

import ml_dtypes

import concourse.bass as bass
import concourse.mybir as mybir
from concourse.bass_utils import run_bass_kernel_spmd
from concourse.tile import TileContext

# Problem constants (hardcoded per the harness contract).
B, S, D, K = 32, 4096, 512, 16
N_CORES = 8
BC = B // N_CORES  # batches per core
P = 128  # SBUF partitions / contraction size
DC = D // P  # d-chunks per position
TN = 512  # positions per matmul (PSUM bank = 512 fp32)
XH = 2048  # positions per x-tile load (SBUF budget)
NH = S // XH
NTH = XH // TN  # matmul tiles per x-tile
PITCH = S + K  # Y scratch row pitch
DIAG = PITCH + 1  # stride that walks the shifted diagonal
YFLAT = K * DIAG  # per-batch scratch elems (incl. rearrange pad)

_F32 = mybir.dt.float32
_BF16 = mybir.dt.bfloat16
BF = ml_dtypes.bfloat16

DEFAULT_MODE = "bf16x1"


def _split_multiwaits(nc, max_waits=1):
    """This container's walrus build accepts at most one sync-wait command
    per instruction ("Too many sync wait commands" in setupSyncWait
    otherwise). Splitting a multi-wait instruction into a chain of
    same-engine single-wait Drains is semantically identical: waits are
    conjunctive and each engine executes its stream in order."""
    n = 0
    for fn in nc.m.functions:
        for blk in fn.blocks:
            out = []
            for ins in blk.instructions:
                si = getattr(ins, "sync_info", None)
                waits = list(si.on_wait) if si is not None and si.on_wait else []
                if len(waits) > max_waits:
                    extra = waits[: len(waits) - max_waits]
                    si.on_wait = waits[len(waits) - max_waits :]
                    for i in range(0, len(extra), max_waits):
                        # EVENT_SEMAPHORE is a pure wait carrier (~20-50 ns);
                        # a Drain here would flush the engine pipeline (on
                        # TensorE that costs microseconds per occurrence).
                        d = mybir.InstEventSemaphore(
                            name=nc.get_next_instruction_name(),
                            engine=ins.engine,
                            ins=[],
                            outs=[],
                            sync_info=mybir.SyncInfo(
                                on_wait=extra[i : i + max_waits], on_update=[]
                            ),
                        )
                        out.append(d)
                        n += 1
                out.append(ins)
            if len(out) != len(blk.instructions):
                blk.instructions = out
    return n


def build_nc_simple(mm_dt):
    """Single-pass variant: one x tensor / one w tensor of dtype mm_dt."""
    nc = bass.Bass("TRN2", debug=False)
    xt = nc.dram_tensor("xt", [BC, D, S], mm_dt, kind="ExternalInput")
    w = nc.dram_tensor("w", [P, DC * K], mm_dt, kind="ExternalInput")
    bias = nc.dram_tensor("bias", [1, 1], _F32, kind="ExternalInput")
    ones_d = nc.dram_tensor("ones", [K, 1], mm_dt, kind="ExternalInput")
    zer_d = nc.dram_tensor("zer", [K, K], mm_dt, kind="ExternalInput")
    out = nc.dram_tensor("out", [BC, S], _F32, kind="ExternalOutput")

    with TileContext(nc) as tc:
        with (
            tc.tile_pool(name="consts", bufs=1) as cpool,
            tc.tile_pool(name="xp", bufs=2) as xpool,
            tc.tile_pool(name="yp", bufs=2) as ypool,
            tc.tile_pool(name="afp", bufs=2) as apool,
            tc.tile_pool(name="obp", bufs=2) as opool,
            tc.tile_pool(name="psy", bufs=2, space="PSUM") as psy,
            tc.tile_pool(name="pso", bufs=2, space="PSUM") as pso,
            tc.tile_pool(name="dscr", bufs=1, space="DRAM") as dpool,
        ):
            wsb = cpool.tile([P, DC * K], mm_dt)
            nc.sync.dma_start(out=wsb[:, :], in_=w[:, :])
            bsb = cpool.tile([1, 1], _F32)
            nc.sync.dma_start(out=bsb[:, :], in_=bias[:, :])
            ones = cpool.tile([K, 1], mm_dt)
            nc.sync.dma_start(out=ones[:, :], in_=ones_d[:, :])
            zer = cpool.tile([K, K], mm_dt)
            nc.sync.dma_start(out=zer[:, :], in_=zer_d[:, :])
            yscr = dpool.tile([BC, YFLAT], mm_dt)

            for b in range(BC):
                tail = yscr[b, 0 : K * PITCH].rearrange("(k r) -> k r", r=PITCH)[
                    :, S:PITCH
                ]
                nc.sync.dma_start(out=tail, in_=zer[:, :])

            for b in range(BC):
                ybuf = ypool.tile([K, S], mm_dt)
                for h in range(NH):
                    xb = xpool.tile([P, DC * XH], mm_dt)
                    nc.sync.dma_start(
                        out=xb[:, :].rearrange("p (dc n) -> p dc n", n=XH),
                        in_=xt[b][:, h * XH : (h + 1) * XH].rearrange(
                            "(dc p) n -> p dc n", p=P
                        ),
                    )
                    for tt in range(NTH):
                        t = h * NTH + tt
                        py = psy.tile([K, TN], _F32)
                        for dc in range(DC):
                            nc.tensor.matmul(
                                py[:, :],
                                wsb[:, dc * K : (dc + 1) * K],
                                xb[:, dc * XH + tt * TN : dc * XH + (tt + 1) * TN],
                                start=(dc == 0),
                                stop=(dc == DC - 1),
                            )
                        nc.vector.tensor_copy(
                            ybuf[:, t * TN : (t + 1) * TN], py[:, :]
                        )

                ywr = yscr[b, 0 : K * PITCH].rearrange("(k r) -> k r", r=PITCH)[
                    :, 0:S
                ]
                nc.sync.dma_start(out=ywr, in_=ybuf[:, :])

                af = apool.tile([K, S], mm_dt)
                ard = yscr[b, :].rearrange("(k r) -> k r", r=DIAG)[:, 0:S]
                nc.sync.dma_start(out=af, in_=ard)

                ob = opool.tile([1, S], _F32)
                for t in range(S // TN):
                    po = pso.tile([1, TN], _F32)
                    nc.tensor.matmul(
                        po[:, :],
                        ones[:, :],
                        af[:, t * TN : (t + 1) * TN],
                        start=True,
                        stop=True,
                    )
                    nc.scalar.add(
                        ob[:, t * TN : (t + 1) * TN], po[:, :], bsb[0:1, 0:1]
                    )
                nc.sync.dma_start(out=out[b : b + 1, :], in_=ob[:, :])

    _split_multiwaits(nc)
    return nc


def build_nc_bf16x3(xh_=2048, xbufs=4):
    """3-pass bf16 split-precision variant (see module docstring).

    Pipelining details (from trace analysis of earlier versions):
      - x is loaded in 1 MB chunks; x-hi on the Sync HWDGE ring, x-lo on
        the Scalar ring; consts / scratch bounce / output go through SWDGE
        (gpsimd) so a waiting scratch DMA never head-of-line blocks the
        next x prefetch (HWDGE triggers are FIFO per ring). Batch 0 opens
        with two small chunks so the PE starts ~5 us earlier.
      - The two xh passes (xh*wh, xh*wl) share the moving operand, so one
        [128, 48] stationary (wh | zeros | wl - the zeros make the Yhl
        rows land 32-aligned) computes both in a single 512-cycle matmul;
        the xl*wh pass accumulates onto the Yhl rows directly.
      - The three Y streams (hi, lo, cross) live in one [96, S] SBUF tile
        at partition offsets 0/32/64, so each scratch bounce is ONE write
        + ONE read DMA: scratch rows are ordered (k, stream) with pitch
        w_, which makes the per-k diagonal shift a linear 3-D access
        pattern (strides 3*w_+1, w_, 1).
      - The scratch round trip has ~4-6 us latency and the PE queue is
        in-order, so stage 2 runs on two sub-ranges: the first is bounced
        after stage-1 tile 3 and consumed after tile 5; the second is
        bounced at batch end and consumed during the NEXT batch.
    """
    xh = xh_
    ntile = S // TN

    nc = bass.Bass("TRN2", debug=False)
    xth = nc.dram_tensor("xth", [BC, D, S], _BF16, kind="ExternalInput")
    xtl = nc.dram_tensor("xtl", [BC, D, S], _BF16, kind="ExternalInput")
    wd = nc.dram_tensor("w", [P, DC * 3 * K], _BF16, kind="ExternalInput")
    bias = nc.dram_tensor("bias", [1, 1], _F32, kind="ExternalInput")
    ones_d = nc.dram_tensor("ones", [3 * K, 1], _BF16, kind="ExternalInput")
    zer_d = nc.dram_tensor("zer", [3 * K, K], _BF16, kind="ExternalInput")
    out = nc.dram_tensor("out", [BC, S], _F32, kind="ExternalOutput")

    # Stage-2 sub-ranges (out columns) and the stage-1 tile after whose
    # evacuation each range's Y data (incl. K-1 lookahead) is complete.
    RANGES = [(0, 3 * TN), (3 * TN, S)]
    READY = [3, ntile - 1]
    G = 3  # streams

    with TileContext(nc) as tc:
        with (
            tc.tile_pool(name="consts", bufs=1) as cpool,
            tc.tile_pool(name="xph", bufs=xbufs) as xpool_h,
            tc.tile_pool(name="xpl", bufs=xbufs) as xpool_l,
            tc.tile_pool(name="ypool", bufs=2) as ypool,
            tc.tile_pool(name="afp", bufs=4) as apool,
            tc.tile_pool(name="obp", bufs=2) as opool,
            tc.tile_pool(name="psy", bufs=4, space="PSUM") as psy,
            tc.tile_pool(name="pso", bufs=3, space="PSUM") as pso,
            tc.tile_pool(name="dscr", bufs=1, space="DRAM") as dpool,
        ):
            wsb = cpool.tile([P, DC * 3 * K], _BF16)
            nc.gpsimd.dma_start(out=wsb[:, :], in_=wd[:, :])
            bsb = cpool.tile([1, 1], _F32)
            nc.gpsimd.dma_start(out=bsb[:, :], in_=bias[:, :])
            ones = cpool.tile([3 * K, 1], _BF16)
            nc.gpsimd.dma_start(out=ones[:, :], in_=ones_d[:, :])
            zer = cpool.tile([3 * K, K], _BF16)
            nc.gpsimd.dma_start(out=zer[:, :], in_=zer_d[:, :])

            # Scratch per (batch, range): rows ordered (k, stream), pitch
            # w_ = hi - lo + K; +K pad for the diagonal view.
            scr = {}
            for b in range(BC):
                for r, (lo, hi) in enumerate(RANGES):
                    w_ = hi - lo + K
                    scr[(b, r)] = dpool.tile(
                        [G * K * (w_ + 1)], _BF16, name=f"scr{r}_{b}"
                    )

            # Zero tails of the last-range scratches (reads past S).
            for b in range(BC):
                lo, hi = RANGES[-1]
                w_ = hi - lo + K
                s = scr[(b, len(RANGES) - 1)]
                v = s[:].rearrange("(g kr) -> g kr", g=G)[
                    :, 0 : K * w_
                ].rearrange("g (k r) -> g k r", r=w_)
                nc.gpsimd.dma_start(out=v[:, :, w_ - K : w_], in_=zer[:, :])

            def bounce(b, r, yb):
                """One write + one read DMA: Y[:, lo:wend) of all three
                streams to scratch rows (k, g), then the k-shifted
                diagonal back as af[48, hi-lo]."""
                lo, hi = RANGES[r]
                w_ = hi - lo + K
                wend = hi + K if r < len(RANGES) - 1 else S
                af = apool.tile([G * K, hi - lo], _BF16, name="af")
                s = scr[(b, r)]
                # g-blocks of pitch K*(w_+1); within a block, row k sits at
                # k*w_, and the k-shifted diagonal is the linear pattern
                # (k*(w_+1) + j) thanks to the +1 block padding... rows:
                # block g holds rows k at pitch w_, padded by K at the end.
                blk = s[:].rearrange("(g kr) -> g kr", g=G)
                rows = blk[:, 0 : K * w_].rearrange("g (k r) -> g k r", r=w_)
                for gi, yrow in enumerate((0, 32, 64)):
                    nc.gpsimd.dma_start(
                        out=rows[gi, :, 0 : wend - lo],
                        in_=yb[yrow : yrow + K, lo:wend],
                    )
                diag = blk[:, 0 : K * (w_ + 1)].rearrange(
                    "g (k r) -> g k r", r=w_ + 1
                )
                nc.gpsimd.dma_start(
                    out=af[:, :], in_=diag[:, :, 0 : hi - lo]
                )
                return af

            def stage2(ob, af, r):
                lo, hi = RANGES[r]
                for t2 in range(lo // TN, hi // TN):
                    po = pso.tile([1, TN], _F32, name="po")
                    j = t2 * TN - lo
                    nc.tensor.matmul(
                        po[:, :],
                        ones[:, :],
                        af[:, j : j + TN],
                        start=True,
                        stop=True,
                    )
                    nc.scalar.add(
                        ob[:, t2 * TN : (t2 + 1) * TN], po[:, :], bsb[0:1, 0:1]
                    )

            pending = None  # deferred stage-2 of the previous batch
            for b in range(BC):
                yb = ypool.tile([3 * 32, S], _BF16)
                ob = opool.tile([1, S], _F32)
                afs = {}
                # b=0 opens with two small chunks (faster first landing).
                if b == 0:
                    chunks = [(0, TN), (TN, TN)]
                    if xh > 2 * TN:
                        chunks.append((2 * TN, xh - 2 * TN))
                    chunks += [(i, xh) for i in range(xh, S, xh)]
                else:
                    chunks = [(i, xh) for i in range(0, S, xh)]
                for c0, cw in chunks:
                    xbh = xpool_h.tile([P, DC * xh], _BF16, name="xbh")
                    nc.sync.dma_start(
                        out=xbh[:, 0 : DC * cw].rearrange(
                            "p (dc n) -> p dc n", n=cw
                        ),
                        in_=xth[b][:, c0 : c0 + cw].rearrange(
                            "(dc p) n -> p dc n", p=P
                        ),
                    )
                    xbl = xpool_l.tile([P, DC * xh], _BF16, name="xbl")
                    nc.scalar.dma_start(
                        out=xbl[:, 0 : DC * cw].rearrange(
                            "p (dc n) -> p dc n", n=cw
                        ),
                        in_=xtl[b][:, c0 : c0 + cw].rearrange(
                            "(dc p) n -> p dc n", p=P
                        ),
                    )
                    for tt in range(cw // TN):
                        t = (c0 + tt * TN) // TN
                        if t == 2 and pending is not None:
                            pending()
                            pending = None
                        py48 = psy.tile([3 * K, TN], _F32, name="py48")
                        for dc in range(DC):
                            xsl = slice(
                                dc * cw + tt * TN, dc * cw + (tt + 1) * TN
                            )
                            nc.tensor.matmul(
                                py48[:, :],
                                wsb[:, dc * 3 * K : (dc + 1) * 3 * K],
                                xbh[:, xsl],
                                start=(dc == 0),
                                stop=False,
                            )
                        for dc in range(DC):
                            # xl*wh accumulates straight onto the Yhl rows
                            # (32-aligned PSUM slice), so no extra adds.
                            xsl = slice(
                                dc * cw + tt * TN, dc * cw + (tt + 1) * TN
                            )
                            nc.tensor.matmul(
                                py48[2 * K : 3 * K, :],
                                wsb[:, dc * 3 * K : dc * 3 * K + K],
                                xbl[:, xsl],
                                start=False,
                                stop=(dc == DC - 1),
                            )
                        # Evacuate: yh = bf16(Yhh), yl = bf16(Yhh - yh),
                        # yc = bf16(Yhl + Ylh), into one [96, S] tile at
                        # partition offsets 0 / 32 / 64.
                        yhs = yb[0:K, t * TN : (t + 1) * TN]
                        nc.vector.tensor_copy(yhs, py48[0:K, :])
                        nc.vector.tensor_tensor(
                            yb[32 : 32 + K, t * TN : (t + 1) * TN],
                            py48[0:K, :],
                            yhs,
                            mybir.AluOpType.subtract,
                        )
                        nc.vector.tensor_copy(
                            yb[64 : 64 + K, t * TN : (t + 1) * TN],
                            py48[2 * K : 3 * K, :],
                        )
                        for r, rdy in enumerate(READY):
                            if t == rdy:
                                afs[r] = bounce(b, r, yb)
                        if t == 5:
                            stage2(ob, afs[0], 0)

                def make_pending(b=b, afs=afs, ob=ob):
                    def emit():
                        stage2(ob, afs[1], 1)
                        nc.gpsimd.dma_start(out=out[b : b + 1, :], in_=ob[:, :])

                    return emit

                pending = make_pending()
            # Last batch: nothing left to hide behind; emit immediately.
            if pending is not None:
                pending()

    _split_multiwaits(nc)
    return nc


def _sort_final_waits(nc):
    """The teardown block ends each engine with one multi-wait instruction
    over ~20 semaphores; _split_multiwaits turns it into a serial chain of
    single-wait instructions (~160 ns each on SP).  If an early link in
    the chain happens to be the LAST-firing semaphore, the engine blocks
    there and then walks the remaining ~18 checks after the kernel's real
    work is done (~3 us of pure epilogue).  Waits are conjunctive, so
    sort each list by the body-order of the semaphore's last update:
    early-finishing sems get checked while the tail is still running and
    only the genuinely last one gates program end."""
    fn = nc.m.functions[0]
    if len(fn.blocks) < 3:
        return
    last_upd = {}
    for i, ins in enumerate(fn.blocks[1].instructions):
        si = getattr(ins, "sync_info", None)
        if si is not None and si.on_update:
            for u in si.on_update:
                uid = getattr(u, "id", None)
                if uid is not None:
                    last_upd[uid] = i
    for ins in fn.blocks[2].instructions:
        si = getattr(ins, "sync_info", None)
        if si is not None and si.on_wait and len(si.on_wait) > 1:
            ws = list(si.on_wait)
            ws.sort(key=lambda w: last_upd.get(w.id, -1))
            si.on_wait = ws


def build_nc_bf16x1(xh_=2048, xbufs=8):
    """Single-stream bf16 variant (~4e-3 rel err, well under the 2e-2 gate).

    Halves HBM traffic vs bf16x3: x is loaded once as bf16 (16.8 MB/core vs
    33.5 MB), which matters because the kernel is DMA-bound (360 GB/s/core,
    PE needs only ~29 us of the ~50 us DMA floor).

    Stage 2 uses a packed diagonal read: scratch rows are written at pitch
    PITCH1 = S+K and read at pitch PITCH1+1 (the k-shift), with each row's
    range [lo, lo+Cr*TN) split into Cr aligned TN-blocks stacked on
    partitions (c, k).  A [Cr*K, Cr] block-selector stationary
    sel[(c,k), o] = (c == o) then sums over k for Cr output blocks in ONE
    512-cycle matmul (4x fewer PE cycles than the ones-vector version).
    """
    xh = xh_
    ntile = S // TN
    PITCH1 = S + K

    nc = bass.Bass("TRN2", debug=False)
    xt = nc.dram_tensor("xt", [BC, D, S], _BF16, kind="ExternalInput")
    wd = nc.dram_tensor("w", [P, DC * K], _BF16, kind="ExternalInput")
    bias = nc.dram_tensor("bias", [8, 1], _F32, kind="ExternalInput")
    sel_d = nc.dram_tensor("sel", [P, ntile], _BF16, kind="ExternalInput")
    zer_d = nc.dram_tensor("zer", [K, K], _BF16, kind="ExternalInput")
    out = nc.dram_tensor("out", [BC, S], _F32, kind="ExternalOutput")

    with TileContext(nc) as tc:
        with (
            tc.tile_pool(name="consts", bufs=1) as cpool,
            tc.tile_pool(name="xp", bufs=xbufs) as xpool,
            tc.tile_pool(name="ypool", bufs=2) as ypool,
            tc.tile_pool(name="afp", bufs=4) as apool,
            tc.tile_pool(name="obp", bufs=4) as opool,
            tc.tile_pool(name="psy", bufs=6, space="PSUM") as psy,
            tc.tile_pool(name="pso", bufs=2, space="PSUM") as pso,
            tc.tile_pool(name="dscr", bufs=1, space="DRAM") as dpool,
        ):
            wsb = cpool.tile([P, DC * K], _BF16)
            nc.gpsimd.dma_start(out=wsb[:, :], in_=wd[:, :])
            bsb = cpool.tile([8, 1], _F32)
            nc.gpsimd.dma_start(out=bsb[:, :], in_=bias[:, :])
            selsb = cpool.tile([P, ntile], _BF16)
            nc.gpsimd.dma_start(out=selsb[:, :], in_=sel_d[:, :])
            zer = cpool.tile([K, K], _BF16)
            nc.gpsimd.dma_start(out=zer[:, :], in_=zer_d[:, :])

            scr = {}
            for b in range(BC):
                scr[b] = dpool.tile([K * (PITCH1 + 1)], _BF16, name=f"scr{b}")
                # Zero the [S, PITCH1) tail of each pitch-row once; the
                # shifted read of row k touches cols [S, S+k).
                wv = scr[b][0 : K * PITCH1].rearrange("(k r) -> k r", r=PITCH1)
                nc.gpsimd.dma_start(out=wv[:, S:PITCH1], in_=zer[:, :])

            def bounce_read(b, box, eng):
                """Whole-batch packed diagonal read: af[(c,k), j'] =
                Y[k, c*TN + j' + k], 8 blocks x 16 k = 128 partitions."""
                af = apool.tile([ntile * K, TN], _BF16, name="af")
                dv = scr[b][:].rearrange("(k r) -> k r", r=PITCH1 + 1)
                src = dv[:, 0 : ntile * TN].rearrange("k (c j) -> c k j", j=TN)
                eng.dma_start(out=af[:, :], in_=src)
                box["af"] = af

            def stage2(b, box):
                af = box["af"]
                po = pso.tile([ntile, TN], _F32, name="po")
                nc.tensor.matmul(
                    po[:, :], selsb[:, :], af[:, :], start=True, stop=True
                )
                ob = opool.tile([ntile, TN], _F32, name="ob")
                nc.scalar.add(ob[:, :], po[:, :], bsb[0:ntile, 0:1])
                # Output on the scalar HWDGE ring: the trigger directly
                # follows the bias-add on the same engine, so it never
                # waits (and SWDGE stays free for the mid-batch bounces).
                nc.scalar.dma_start(
                    out=out[b, :].rearrange("(c j) -> c j", j=TN),
                    in_=ob[:, :],
                )

            rings = [nc.sync, nc.scalar]
            ring_i = 0
            pending = None  # deferred bounce-read + stage-2 of previous batch
            for b in range(BC):
                yb = ypool.tile([K, S], _BF16)
                if b == 0:
                    # Ramp: small chunks so the PE starts early.
                    chunks = [(0, TN), (TN, TN), (2 * TN, 2 * TN)]
                    chunks += [(i, xh) for i in range(2 * TN * 2, S, xh)]
                elif b == BC - 1:
                    # Taper: small final chunks so the last tiles start
                    # right as the x stream ends (shorter drain).
                    chunks = [(0, xh), (xh, 2 * TN), (xh + 2 * TN, TN),
                              (xh + 3 * TN, TN)]
                else:
                    chunks = [(i, xh) for i in range(0, S, xh)]
                for c0, cw in chunks:
                    xb = xpool.tile([P, DC * xh], _BF16, name="xb")
                    rings[ring_i % len(rings)].dma_start(
                        out=xb[:, 0 : DC * cw].rearrange(
                            "p (dc n) -> p dc n", n=cw
                        ),
                        in_=xt[b][:, c0 : c0 + cw].rearrange(
                            "(dc p) n -> p dc n", p=P
                        ),
                    )
                    ring_i += 1
                    for tt in range(cw // TN):
                        t = (c0 + tt * TN) // TN
                        if t == 1 and pending is not None:
                            pending["read"](nc.gpsimd)
                        # The last batch consumes the previous batch's
                        # stage-2 at t==5, so its bias-add/out never sit
                        # behind the tail bounce triggers on the scalar
                        # engine; mid-run batches keep the roomier t==7.
                        tcons = 5 if b == BC - 1 else 7
                        if t == tcons and pending is not None:
                            pending["stage2"]()
                            pending = None
                        py = psy.tile([K, TN], _F32, name="py")
                        for dc in range(DC):
                            xsl = slice(
                                dc * cw + tt * TN, dc * cw + (tt + 1) * TN
                            )
                            nc.tensor.matmul(
                                py[:, :],
                                wsb[:, dc * K : (dc + 1) * K],
                                xb[:, xsl],
                                start=(dc == 0),
                                stop=(dc == DC - 1),
                            )
                        nc.vector.tensor_copy(
                            yb[:, t * TN : (t + 1) * TN], py[:, :]
                        )
                        wv = scr[b][0 : K * PITCH1].rearrange(
                            "(k r) -> k r", r=PITCH1
                        )
                        if b == BC - 1 and t == 6:
                            # Last batch: pre-write cols [0, 3088) on the
                            # now-idle scalar HWDGE ring so the post-t7
                            # write is tiny.
                            nc.scalar.dma_start(
                                out=wv[:, 0 : 6 * TN + K],
                                in_=yb[:, 0 : 6 * TN + K],
                            )
                        if t == ntile - 1:
                            if b == BC - 1:
                                nc.scalar.dma_start(
                                    out=wv[:, 6 * TN + K : S],
                                    in_=yb[:, 6 * TN + K : S],
                                )
                            else:
                                # Whole-batch scratch write on the (idle)
                                # SWDGE ring; consumed early next batch.
                                nc.gpsimd.dma_start(
                                    out=wv[:, 0:S], in_=yb[:, :]
                                )

                def make_pending(b=b):
                    box = {}
                    return {
                        "read": lambda eng: bounce_read(b, box, eng),
                        "stage2": lambda: stage2(b, box),
                    }

                pending = make_pending()
            if pending is not None:
                pending["read"](nc.sync)
                pending["stage2"]()

    _sort_final_waits(nc)
    _split_multiwaits(nc)
    return nc


_NC_CACHE = {}


def _get_nc(mode):
    if mode not in _NC_CACHE:
        if mode == "bf16x1":
            _NC_CACHE[mode] = build_nc_bf16x1()
        elif mode == "bf16x3":
            _NC_CACHE[mode] = build_nc_bf16x3()
        elif mode == "f32r":
            _NC_CACHE[mode] = build_nc_simple(mybir.dt.float32r)
        elif mode == "f32":
            _NC_CACHE[mode] = build_nc_simple(mybir.dt.float32)
        else:
            raise ValueError(mode)
    return _NC_CACHE[mode]


def _prep_in_maps(embedded, filt, bias, mode):
    embedded = np.ascontiguousarray(embedded, dtype=np.float32)
    filt = np.ascontiguousarray(filt, dtype=np.float32)
    bias = np.ascontiguousarray(bias, dtype=np.float32)
    b11 = bias.reshape(1, 1)

    def wl_layout(f):
        # [p, dc*K + k] = w[k, dc*128 + p]
        return np.ascontiguousarray(
            f.reshape(K, DC, P).transpose(2, 1, 0).reshape(P, DC * K)
        )

    in_maps = []
    if mode == "bf16x1":
        wl = wl_layout(filt.astype(BF).astype(np.float32)).astype(BF)
        ntile = S // TN
        sel = np.zeros((P, ntile), dtype=BF)
        for c in range(ntile):
            sel[c * K : (c + 1) * K, c] = 1
        zer16 = np.zeros((K, K), dtype=BF)
        b8 = np.broadcast_to(bias.reshape(1, 1), (8, 1)).astype(np.float32)
        b8 = np.ascontiguousarray(b8)
        xh = embedded.astype(BF)
        for c in range(N_CORES):
            sl = slice(c * BC, (c + 1) * BC)
            xtc = np.ascontiguousarray(xh[sl].transpose(0, 2, 1))
            in_maps.append(
                {"xt": xtc, "w": wl, "bias": b8, "sel": sel, "zer": zer16}
            )
    elif mode == "bf16x3":
        wh = filt.astype(BF)
        wlo = (filt - wh.astype(np.float32)).astype(BF)
        whl = wl_layout(wh.astype(np.float32)).reshape(P, DC, K)
        wll = wl_layout(wlo.astype(np.float32)).reshape(P, DC, K)
        # per dc block: [wh (16) | zeros (16) | wl (16)]
        wcat = np.zeros((P, DC, 3 * K), dtype=np.float32)
        wcat[:, :, 0:K] = whl
        wcat[:, :, 2 * K : 3 * K] = wll
        wcat = wcat.reshape(P, DC * 3 * K).astype(BF)
        ones16 = np.ones((3 * K, 1), dtype=BF)
        zer16 = np.zeros((3 * K, K), dtype=BF)
        xh = embedded.astype(BF)
        xl = (embedded - xh.astype(np.float32)).astype(BF)
        for c in range(N_CORES):
            sl = slice(c * BC, (c + 1) * BC)
            xthc = np.ascontiguousarray(xh[sl].transpose(0, 2, 1))
            xtlc = np.ascontiguousarray(xl[sl].transpose(0, 2, 1))
            in_maps.append(
                {
                    "xth": xthc,
                    "xtl": xtlc,
                    "w": wcat,
                    "bias": b11,
                    "ones": ones16,
                    "zer": zer16,
                }
            )
    else:
        wl = wl_layout(filt)
        ones16 = np.ones((K, 1), dtype=np.float32)
        zer16 = np.zeros((K, K), dtype=np.float32)
        for c in range(N_CORES):
            xc = embedded[c * BC : (c + 1) * BC]
            xtc = np.ascontiguousarray(xc.transpose(0, 2, 1))
            in_maps.append(
                {"xt": xtc, "w": wl, "bias": b11, "ones": ones16, "zer": zer16}
            )
    return in_maps


def run(embedded, filt, bias, mode=DEFAULT_MODE, trace=False, **spmd_kwargs):
    nc = _get_nc(mode)
    in_maps = _prep_in_maps(embedded, filt, bias, mode)
    res = run_bass_kernel_spmd(
        nc, in_maps, list(range(N_CORES)), trace=trace, **spmd_kwargs
    )
    out = np.concatenate([res.results[c]["out"] for c in range(N_CORES)], axis=0)
    return out.astype(np.float32), res


def kernel(embedded, filt, bias):
    out, _ = run(embedded, filt, bias)
    return out

